# revision 16
# baseline (speedup 1.0000x reference)
"""Kalman filter + RTS smoother kernel for Trainium2 (8 NeuronCores).

T=512 steps, B=512 batch, L=8 latent, O=16 obs. Data-parallel over B
(64/core); on each core the T axis is split into 16 chunks (2 on the
partition axis x 8 on the free axis, V=32 steps each) that run
concurrently, each with a W-step discarded warmup exploiting the
exponential forgetting (~0.4/step) of the Kalman recursions.

The 8 free-axis chunks are split into two independent instruction
streams: stream A's covariance chain runs on the Vector engine (DVE,
fp16 2x fast path), stream B's on the GpSimd/Pool engine, so the two
serial recursions pipeline against each other instead of serializing
on one engine. All 8x8 matmuls are emitted in ABt broadcast form
(unit innermost strides on both operands, exploiting the symmetry of
the covariance operands) with add-fold trees instead of TensorReduce.
The filter solve is a batched no-pivot Gauss-Jordan on the augmented
system [I + Sig G | Sig | Sig v] (divide ALU op, no reciprocal), which
yields Sig_z and the mu update in one elimination. All staging DRAM
(G/A/cy and the smoother inputs) is fp16 in h-major planes so every
per-step load/store is a single DMA.
"""
import sys
import numpy as np

for _p in ("/opt/trn_rl_repo",):
    if _p not in sys.path:
        sys.path.append(_p)

import concourse.bass as bass
import concourse.bacc as bacc
import concourse.mybir as mybir
from concourse.tile import TileContext
from concourse import bass_utils

F32 = mybir.dt.float32
F16 = mybir.dt.float16
AX = mybir.AxisListType
OP = mybir.AluOpType

L = 8
O = 16
LL = L * L
NB = 64
NCORE = 8
FW = 2 * LL + L          # fwd staging row: [G | A | cy] = 136
SW = 2 * (LL + L)        # smoother staging row: [Sf | muf | Sp(t+1) | mup(t+1)] = 144
PW = L * (L + 1)         # packed output row = 72


def build_kernel(T=512, NFA=8, NFB=0, W=8, TB=16,
                 pack_a="pool", pack_b="dve"):
    NF = NFA + NFB
    TH = T // 2
    V = TH // NF
    assert NF * V == TH and V >= W
    NSTEP = V + W
    ULEN = TH + 2 * W + 1
    SLEN = TH + 2 * W + 1

    nc = bacc.Bacc("TRN2", target_bir_lowering=False, debug=False,
                   num_devices=NCORE)

    a_in = nc.dram_tensor("a_in", (2, NB, TH, LL), F32, kind="ExternalInput")
    c_in = nc.dram_tensor("c_in", (2, NB, TH, O * L), F32, kind="ExternalInput")
    y_in = nc.dram_tensor("y_in", (2, NB, TH, O), F32, kind="ExternalInput")
    cons = nc.dram_tensor("cons", (128, 2 * LL), F16, kind="ExternalInput")
    csig = nc.dram_tensor("csig", (128, NFA * LL), F16, kind="ExternalInput")
    cmu = nc.dram_tensor("cmu", (128, NFA * L), F16, kind="ExternalInput")
    out = nc.dram_tensor("out", (T, NB, L, L + 1), F32, kind="ExternalOutput")

    FWD = nc.dram_tensor("fwd", (2, NB, ULEN, FW), F16, kind="Internal")
    SPFM = nc.dram_tensor("spfm", (2, NB, SLEN, SW), F16, kind="Internal")
    fwdv = FWD.ap().rearrange("h b u e -> (h b) u e")
    spfmv = SPFM.ap().rearrange("h b u e -> (h b) u e")

    VE = None  # set inside context

    def v4(flat_ap, nfx):
        return flat_ap.rearrange("p (f i k) -> p f i k", f=nfx, i=L, k=L)

    with TileContext(nc) as tc:
        with tc.tile_pool(name="const", bufs=1) as cpool:
            VE, PO = nc.vector, nc.gpsimd

            def eng(name):
                return VE if name == "dve" else PO

            # ---------------- constants ----------------
            CT = cpool.tile([128, 2 * LL], F16)
            nc.sync.dma_start(CT[:], cons.ap())
            Qv = CT[:, 0:LL].rearrange("p (i k) -> p i k", i=L, k=L)
            Iv = CT[:, LL:2 * LL].rearrange("p (i k) -> p i k", i=L, k=L)
            CS = cpool.tile([128, NFA * LL], F16)
            CM = cpool.tile([128, NFA * L], F16)
            nc.sync.dma_start(CS[:], csig.ap())
            nc.sync.dma_start(CM[:], cmu.ap())

            # ---------------- DRAM pads ----------------
            # FWD h=0 plane rows [0, W): zeros (chunk-0 warmup reads them)
            ZP = cpool.tile([128, W * FW], F16)
            VE.memset(ZP[:], 0.0)
            nc.sync.dma_start(
                FWD.ap()[0, :, 0:W, :],
                ZP[0:NB, :].rearrange("b (u e) -> b u e", u=W, e=FW))
            # FWD both planes row range [TH+W, TH+2W]: zero A pad for h=1
            # (t >= T => J=0 boundary); h=0 range is overwritten by the
            # dram-dram dup below.
            ZP2 = cpool.tile([128, (W + 1) * FW], F16)
            VE.memset(ZP2[:], 0.0)
            for h in range(2):
                nc.sync.dma_start(
                    FWD.ap()[h, :, TH + W:TH + 2 * W + 1, :],
                    ZP2[0:NB, :].rearrange("b (u e) -> b u e", u=W + 1, e=FW))
            # SPFM h=1 tail pad rows [TH+W, TH+2W]: [I | 0 | I | 0]
            SPAD = cpool.tile([128, (W + 1) * SW], F16)
            VE.memset(SPAD[:], 0.0)
            spadv = SPAD[:].rearrange("p (u e) -> p u e", u=W + 1, e=SW)
            for j in range(W + 1):
                PO.tensor_copy(spadv[:, j, 0:LL]
                               .rearrange("p (i k) -> p i k", i=L, k=L), Iv)
                PO.tensor_copy(spadv[:, j, LL + L:2 * LL + L]
                               .rearrange("p (i k) -> p i k", i=L, k=L), Iv)
            nc.sync.dma_start(
                SPFM.ap()[1, :, TH + W:TH + 2 * W + 1, :],
                SPAD[64:128, :].rearrange("b (u e) -> b u e", u=W + 1, e=SW))

            # ---------------- prep: G = C^T C, cy = C^T y, stage A ----------
            with tc.tile_pool(name="prep", bufs=2) as ppool:
                NIT = TH // TB
                for j in range(NIT):
                    E = VE if (j % 4) != 3 else PO
                    DQ = nc.sync
                    At = ppool.tile([128, TB * LL], F32, tag="pA")
                    Cr = ppool.tile([128, TB * O * L], F32, tag="pC")
                    Yr = ppool.tile([128, TB * O], F32, tag="pY")
                    DQ.dma_start(
                        At[:].rearrange("p (u e) -> p u e", u=TB, e=LL),
                        a_in.ap().rearrange("h b u e -> (h b) u e")
                        [:, j * TB:(j + 1) * TB, :])
                    DQ.dma_start(
                        Cr[:].rearrange("p (u e) -> p u e", u=TB, e=O * L),
                        c_in.ap().rearrange("h b u e -> (h b) u e")
                        [:, j * TB:(j + 1) * TB, :])
                    DQ.dma_start(
                        Yr[:].rearrange("p (u e) -> p u e", u=TB, e=O),
                        y_in.ap().rearrange("h b u e -> (h b) u e")
                        [:, j * TB:(j + 1) * TB, :])
                    PKF = ppool.tile([128, TB * FW], F16, tag="pF")
                    pkv = PKF[:].rearrange("p (u e) -> p u e", u=TB, e=FW)
                    # cast A into pack
                    E.tensor_copy(pkv[:, :, LL:2 * LL],
                                  At[:].rearrange("p (u e) -> p u e",
                                                  u=TB, e=LL))
                    # C^T (transposed cast): (u, l, o) <- (u, o, l)
                    CtT = ppool.tile([128, TB * O * L], F16, tag="pCt")
                    ctv = CtT[:].rearrange("p (u l o) -> p u l o",
                                           u=TB, l=L, o=O)
                    E.tensor_copy(ctv,
                                  Cr[:].rearrange("p (u o l) -> p u o l",
                                                  u=TB, o=O, l=L)
                                  .rearrange("p u o l -> p u l o"))
                    Y16 = ppool.tile([128, TB * O], F16, tag="pY16")
                    E.tensor_copy(Y16[:], Yr[:])
                    # G mults: per u, (l, m, o) = Ct[l,o] * Ct[m,o]
                    GT = ppool.tile([128, TB * L * L * O], F16, tag="pG")
                    gtv = GT[:].rearrange("p (u l m o) -> p u l m o",
                                          u=TB, l=L, m=L, o=O)
                    for u in range(TB):
                        E.tensor_tensor(
                            gtv[:, u],
                            ctv[:, u].unsqueeze(2).broadcast_to((128, L, L, O)),
                            ctv[:, u].unsqueeze(1).broadcast_to((128, L, L, O)),
                            OP.mult)
                    gk = GT[:].rearrange("p (x o) -> p x o", o=O)
                    E.tensor_tensor(gk[:, :, 0:8], gk[:, :, 0:8],
                                    gk[:, :, 8:16], OP.add)
                    E.tensor_tensor(gk[:, :, 0:4], gk[:, :, 0:4],
                                    gk[:, :, 4:8], OP.add)
                    E.tensor_tensor(gk[:, :, 0:2], gk[:, :, 0:2],
                                    gk[:, :, 2:4], OP.add)
                    gkv = GT[:].rearrange("p (u x o) -> p u x o",
                                          u=TB, x=LL, o=O)
                    E.tensor_tensor(pkv[:, :, 0:LL],
                                    gkv[:, :, :, 0], gkv[:, :, :, 1], OP.add)
                    # cy: (u, l, o) = Ct[l,o] * y[o]
                    CYT = ppool.tile([128, TB * L * O], F16, tag="pcy")
                    cyv = CYT[:].rearrange("p (u l o) -> p u l o",
                                           u=TB, l=L, o=O)
                    E.tensor_tensor(
                        cyv, ctv,
                        Y16[:].rearrange("p (u o) -> p u o", u=TB, o=O)
                        .unsqueeze(2).broadcast_to((128, TB, L, O)),
                        OP.mult)
                    ck = CYT[:].rearrange("p (x o) -> p x o", o=O)
                    E.tensor_tensor(ck[:, :, 0:8], ck[:, :, 0:8],
                                    ck[:, :, 8:16], OP.add)
                    E.tensor_tensor(ck[:, :, 0:4], ck[:, :, 0:4],
                                    ck[:, :, 4:8], OP.add)
                    E.tensor_tensor(ck[:, :, 0:2], ck[:, :, 0:2],
                                    ck[:, :, 2:4], OP.add)
                    ckv = CYT[:].rearrange("p (u l o) -> p u l o",
                                           u=TB, l=L, o=O)
                    E.tensor_tensor(pkv[:, :, 2 * LL:2 * LL + L],
                                    ckv[:, :, :, 0], ckv[:, :, :, 1], OP.add)
                    DQ.dma_start(
                        fwdv[:, W + j * TB:W + (j + 1) * TB, :], pkv)

            # dram-dram dups across the h boundary:
            # FWD h=1 front pad [0, W) <- h=0 rows [TH, TH+W)
            nc.sync.dma_start(FWD.ap()[1, :, 0:W, :],
                              FWD.ap()[0, :, TH:TH + W, :])
            # FWD h=0 tail rows [TH+W, TH+2W] <- h=1 rows [W, 2W]
            nc.sync.dma_start(FWD.ap()[0, :, TH + W:TH + 2 * W + 1, :],
                              FWD.ap()[1, :, W:2 * W + 1, :])

            # ---------------- helper emitters ----------------
            def mm_abt(E, dstv, tmp, x4, y4, nfx):
                """dst[f,a,c] = sum_k x4[f,a,k] * y4[f,c,k]."""
                tv = tmp[:].rearrange("p (f a c k) -> p f a c k",
                                      f=nfx, a=L, c=L, k=L)
                for i in range(nfx):
                    E.tensor_tensor(
                        tv[:, i],
                        x4[:, i].unsqueeze(2).broadcast_to((128, L, L, L)),
                        y4[:, i].unsqueeze(1).broadcast_to((128, L, L, L)),
                        OP.mult)
                tk = tmp[:, 0:nfx * 512].rearrange("p (x k) -> p x k", k=L)
                E.tensor_tensor(tk[:, :, 0:4], tk[:, :, 0:4],
                                tk[:, :, 4:8], OP.add)
                E.tensor_tensor(tk[:, :, 0:2], tk[:, :, 0:2],
                                tk[:, :, 2:4], OP.add)
                E.tensor_tensor(
                    dstv,
                    tk[:, :, 0].rearrange("p (f a c) -> p f a c",
                                          f=nfx, a=L, c=L),
                    tk[:, :, 1].rearrange("p (f a c) -> p f a c",
                                          f=nfx, a=L, c=L),
                    OP.add)

            def mv(E, dst3, tmp, x4, v3, nfx, kind="Av"):
                """dst[f,a] = sum_k X[f,a,k] v[f,k] (Av) or X[f,k,a] v[f,k].
                v3 is a (p, f, k) view."""
                xv = x4 if kind == "Av" else x4.rearrange("p f k l -> p f l k")
                vv = v3.unsqueeze(2).broadcast_to((128, nfx, L, L))
                tv = tmp[:, 0:nfx * LL].rearrange("p (f a k) -> p f a k",
                                                  f=nfx, a=L, k=L)
                E.tensor_tensor(tv, xv, vv, OP.mult)
                E.tensor_tensor(tv[:, :, :, 0:4], tv[:, :, :, 0:4],
                                tv[:, :, :, 4:8], OP.add)
                E.tensor_tensor(tv[:, :, :, 0:2], tv[:, :, :, 0:2],
                                tv[:, :, :, 2:4], OP.add)
                E.tensor_tensor(dst3, tv[:, :, :, 0], tv[:, :, :, 1], OP.add)

            def gj(E, augv, prt, t2t, rct, nfx, wid):
                prtv = prt[:].rearrange("p (r f j) -> p r f j",
                                        r=2, f=nfx, j=wid - 1)
                rctv = rct[:].rearrange("p (r f j) -> p r f j",
                                        r=2, f=nfx, j=1)
                pend = None
                for pp in range(L):
                    jw = wid - 1 - pp
                    prv = prtv[:, pp % 2][:, :, 0:jw]
                    rcv = rctv[:, pp % 2]
                    with nc.allow_low_precision(
                            reason="fp16 pivot reciprocal, pivots are O(1)"):
                        VE.reciprocal(rcv, augv[:, :, pp, pp:pp + 1])
                    E.tensor_tensor(
                        prv, augv[:, :, pp, pp + 1:],
                        rcv.broadcast_to((128, nfx, jw)), OP.mult)
                    if pend is not None:
                        E.tensor_copy(augv[:, :, pend[0], pend[0] + 1:],
                                      pend[1])
                    t2v = t2t[:].rearrange("p (f i j) -> p f i j",
                                           f=nfx, i=L, j=wid - 1)[:, :, :, 0:jw]
                    E.tensor_tensor(
                        t2v,
                        augv[:, :, :, pp:pp + 1]
                        .broadcast_to((128, nfx, L, jw)),
                        prv.unsqueeze(2).broadcast_to((128, nfx, L, jw)),
                        OP.mult)
                    E.tensor_tensor(augv[:, :, :, pp + 1:],
                                    augv[:, :, :, pp + 1:], t2v, OP.subtract)
                    pend = (pp, prv)
                E.tensor_copy(augv[:, :, L - 1, L:], pend[1])

            # ---------------- stream descriptors ----------------
            # Each stream is fully self-contained on one compute engine and
            # one DMA-issue queue (ACT / PE sequencers are otherwise idle),
            # so the two serial recursions never cross-stall. Loads run two
            # steps ahead of the recursion.
            streams = [
                dict(nfx=NFA, off=0, E=VE, Eo=PO, name="a", DQ=nc.sync),
            ]

            with tc.tile_pool(name="work", bufs=3) as pool, \
                 tc.tile_pool(name="work2", bufs=2) as pool2:

                def ft_load(st, s):
                    nfx, off = st["nfx"], st["off"]
                    FT = pool.tile([128, nfx * FW], F16,
                                   tag="FT" + st["name"],
                                   name="FT" + st["name"], bufs=4)
                    st["DQ"].dma_start(
                        FT[:].rearrange("p (f e) -> p f e", f=nfx, e=FW),
                        fwdv[:, off * V + s:
                             off * V + s + (nfx - 1) * V + 1:V, :])
                    st["ftq"].append(FT)

                # ============ FORWARD FILTER ============
                for st in streams:
                    nfx = st["nfx"]
                    st["SIG"] = cpool.tile([128, nfx * LL], F16,
                                           name="SIG0" + st["name"])
                    st["MU"] = cpool.tile([128, nfx * L], F16,
                                          name="MU0" + st["name"])
                    st["E"].memset(st["SIG"][:], 0.0)
                    st["E"].memset(st["MU"][:], 0.0)
                    st["ftq"] = []
                    ft_load(st, 0)
                    ft_load(st, 1)

                for s in range(NSTEP):
                    for st in streams:
                        nfx = st["nfx"]
                        if s + 2 < NSTEP:
                            ft_load(st, s + 2)
                        FT = st["ftq"].pop(0)
                        ftv = FT[:].rearrange("p (f e) -> p f e", f=nfx, e=FW)
                        st["Gt4"] = ftv[:, :, 0:LL].rearrange(
                            "p f (i k) -> p f i k", i=L, k=L)
                        st["At4"] = ftv[:, :, LL:2 * LL].rearrange(
                            "p f (i k) -> p f i k", i=L, k=L)
                        st["CYf"] = ftv[:, :, 2 * LL:2 * LL + L]

                    if s == W:
                        stA = streams[0]
                        VE.tensor_tensor(stA["SIG"][:], stA["SIG"][:], CS[:],
                                         OP.add)
                        VE.tensor_tensor(stA["MU"][:], stA["MU"][:], CM[:],
                                         OP.add)

                    for st in streams:
                        nfx = st["nfx"]
                        nm = st["name"]
                        st["AUG"] = pool2.tile([128, nfx * L * 17], F16,
                                               tag="AUG" + nm, name="AUG" + nm)
                        st["augv"] = st["AUG"][:].rearrange(
                            "p (f i j) -> p f i j", f=nfx, i=L, j=17)
                        st["TMP"] = pool2.tile([128, nfx * 512], F16,
                                               tag="TMP" + nm, name="TMP" + nm)
                        st["PR"] = pool.tile([128, nfx * 2 * 16], F16,
                                             tag="PR" + nm, name="PR" + nm)
                        st["T2"] = pool2.tile([128, nfx * L * 16], F16,
                                              tag="T2" + nm, name="T2" + nm)
                        st["MT"] = pool.tile([128, nfx * LL], F16,
                                             tag="MT" + nm, name="MT" + nm)
                        st["RC"] = pool.tile([128, nfx * 2], F16,
                                             tag="RC" + nm, name="RC" + nm)

                    # aug assembly: cols 8:16 = Sig, col 16 = Sig cy + mu
                    # (mu_z = M^-1 (Sig cy + mu) -- push-through identity)
                    for st in streams:
                        nfx, Eo = st["nfx"], st["Eo"]
                        sig4 = v4(st["SIG"][:], nfx)
                        Eo.tensor_copy(st["augv"][:, :, :, L:2 * L], sig4)
                        mv(Eo, st["augv"][:, :, :, 16], st["MT"], sig4,
                           st["CYf"], nfx)
                        Eo.tensor_tensor(
                            st["augv"][:, :, :, 16],
                            st["augv"][:, :, :, 16],
                            st["MU"][:].rearrange("p (f a) -> p f a",
                                                  f=nfx, a=L), OP.add)

                    # chain: SG mm -> aug cols 0:8, then +I
                    for st in streams:
                        nfx, E = st["nfx"], st["E"]
                        mm_abt(E, st["augv"][:, :, :, 0:L], st["TMP"],
                               v4(st["SIG"][:], nfx), st["Gt4"], nfx)
                        E.tensor_tensor(
                            st["augv"][:, :, :, 0:L],
                            st["augv"][:, :, :, 0:L],
                            Iv.unsqueeze(1).broadcast_to((128, nfx, L, L)),
                            OP.add)

                    # chain: GJ -> [I | Sig_z | mu_z]
                    for st in streams:
                        gj(st["E"], st["augv"], st["PR"], st["T2"],
                           st["RC"], st["nfx"], 17)

                    # mu' = A mu_z
                    for st in streams:
                        nfx, Eo = st["nfx"], st["Eo"]
                        MUn = pool.tile([128, nfx * L], F16,
                                        tag="MUn" + st["name"],
                                        name="MUn" + st["name"])
                        mv(Eo, MUn[:].rearrange("p (f a) -> p f a",
                                                f=nfx, a=L),
                           st["MT"], st["At4"], st["augv"][:, :, :, 16], nfx)
                        st["MUn"] = MUn

                    # chain: ASZ = A Sigz, SIG' = ASZ A^T + Q
                    for st in streams:
                        nfx, E = st["nfx"], st["E"]
                        sigz4 = st["augv"][:, :, :, L:2 * L]
                        ASZ = pool.tile([128, nfx * LL], F16,
                                        tag="ASZ" + st["name"],
                                        name="ASZ" + st["name"])
                        mm_abt(E, v4(ASZ[:], nfx), st["TMP"], st["At4"],
                               sigz4, nfx)
                        SIGn = pool.tile([128, nfx * LL], F16,
                                         tag="SIGn" + st["name"],
                                         name="SIGn" + st["name"])
                        mm_abt(E, v4(SIGn[:], nfx), st["TMP"],
                               v4(ASZ[:], nfx), st["At4"], nfx)
                        E.tensor_tensor(
                            v4(SIGn[:], nfx), v4(SIGn[:], nfx),
                            Qv.unsqueeze(1).broadcast_to((128, nfx, L, L)),
                            OP.add)
                        st["SIGn"] = SIGn

                    # pack [Sf | muf | Sp(t+1) | mup(t+1)] and store
                    if s >= W:
                        for st in streams:
                            nfx, off, E = st["nfx"], st["off"], st["Eo"]
                            PACK = pool.tile([128, nfx * SW], F16,
                                             tag="PK" + st["name"],
                                             name="PK" + st["name"])
                            pkv = PACK[:].rearrange("p (f e) -> p f e",
                                                    f=nfx, e=SW)
                            E.tensor_copy(
                                pkv[:, :, 0:LL].rearrange(
                                    "p f (i k) -> p f i k", i=L, k=L),
                                st["augv"][:, :, :, L:2 * L])
                            E.tensor_copy(pkv[:, :, LL:LL + L],
                                          st["augv"][:, :, :, 16])
                            E.tensor_copy(
                                pkv[:, :, LL + L:2 * LL + L].rearrange(
                                    "p f (i k) -> p f i k", i=L, k=L),
                                v4(st["SIGn"][:], nfx))
                            E.tensor_copy(
                                pkv[:, :, 2 * LL + L:SW],
                                st["MUn"][:].rearrange("p (f a) -> p f a",
                                                       f=nfx, a=L))
                            st["DQ"].dma_start(
                                spfmv[:, off * V + s:
                                      off * V + s + (nfx - 1) * V + 1:V, :],
                                pkv)
                            if st["off"] == 0 and W <= s <= 2 * W:
                                st["DQ"].dma_start(
                                    SPFM.ap()[0, :, TH + s, :],
                                    PACK[64:128, 0:SW])

                    for st in streams:
                        st["SIG"] = st["SIGn"]
                        st["MU"] = st["MUn"]

                # ============ BACKWARD SMOOTHER ============
                def bwd_load(st, r):
                    nfx, off, nm = st["nfx"], st["off"], st["name"]
                    SFT = pool.tile([128, nfx * SW], F16, tag="SF" + nm,
                                    name="SF" + nm, bufs=4)
                    row = off * V + V + 2 * W - 1 - r
                    st["DQ"].dma_start(
                        SFT[:].rearrange("p (f e) -> p f e", f=nfx, e=SW),
                        spfmv[:, row:row + (nfx - 1) * V + 1:V, :])
                    At1 = pool.tile([128, nfx * LL], F16, tag="Ab" + nm,
                                    name="Ab" + nm, bufs=4)
                    st["DQ"].dma_start(
                        At1[:].rearrange("p (f e) -> p f e", f=nfx, e=LL),
                        fwdv[:, row + 1:row + 1 + (nfx - 1) * V + 1:V,
                             LL:2 * LL])
                    st["bq"].append((SFT, At1))

                for st in streams:
                    nfx, off = st["nfx"], st["off"]
                    INIT = cpool.tile([128, nfx * (LL + L)], F16,
                                      name="INIT" + st["name"])
                    st["DQ"].dma_start(
                        INIT[:].rearrange("p (f e) -> p f e",
                                          f=nfx, e=LL + L),
                        spfmv[:, off * V + V + 2 * W:
                              off * V + V + 2 * W + (nfx - 1) * V + 1:V,
                              0:LL + L])
                    iv = INIT[:].rearrange("p (f e) -> p f e", f=nfx, e=LL + L)
                    st["SIGSv"] = iv[:, :, 0:LL].rearrange(
                        "p f (i k) -> p f i k", i=L, k=L)
                    st["MUSv"] = iv[:, :, LL:LL + L]
                    st["bq"] = []
                    bwd_load(st, 0)
                    bwd_load(st, 1)

                for r in range(NSTEP):
                    for st in streams:
                        nfx = st["nfx"]
                        nm = st["name"]
                        if r + 2 < NSTEP:
                            bwd_load(st, r + 2)
                        SFT, At1 = st["bq"].pop(0)
                        sfv = SFT[:].rearrange("p (f e) -> p f e", f=nfx, e=SW)
                        st["Sf4"] = sfv[:, :, 0:LL].rearrange(
                            "p f (i k) -> p f i k", i=L, k=L)
                        st["muf"] = sfv[:, :, LL:LL + L]
                        st["Sp4"] = sfv[:, :, LL + L:2 * LL + L].rearrange(
                            "p f (i k) -> p f i k", i=L, k=L)
                        st["mup"] = sfv[:, :, 2 * LL + L:SW]
                        st["At4"] = v4(At1[:], nfx)

                        st["AUG"] = pool2.tile([128, nfx * L * 16], F16,
                                               tag="AUG" + nm,
                                               name="AUGb" + nm)
                        st["augv"] = st["AUG"][:].rearrange(
                            "p (f i j) -> p f i j", f=nfx, i=L, j=16)
                        st["TMP"] = pool2.tile([128, nfx * 512], F16,
                                               tag="TMP" + nm,
                                               name="TMPb" + nm)
                        st["PR"] = pool.tile([128, nfx * 2 * 15], F16,
                                             tag="PR" + nm, name="PRb" + nm)
                        st["T2"] = pool2.tile([128, nfx * L * 15], F16,
                                              tag="T2" + nm, name="T2b" + nm)
                        st["MT"] = pool.tile([128, nfx * LL], F16,
                                             tag="MT" + nm, name="MTb" + nm)
                        st["RC"] = pool.tile([128, nfx * 2], F16,
                                             tag="RC" + nm, name="RCb" + nm)

                    # DS, DM, aug Sp copy
                    for st in streams:
                        nfx, E = st["nfx"], st["Eo"]
                        DS = pool.tile([128, nfx * LL], F16,
                                       tag="DS" + st["name"],
                                       name="DS" + st["name"])
                        E.tensor_tensor(v4(DS[:], nfx), st["SIGSv"],
                                        st["Sp4"], OP.subtract)
                        st["DS4"] = v4(DS[:], nfx)
                        DM = pool.tile([128, nfx * L], F16,
                                       tag="DM" + st["name"],
                                       name="DM" + st["name"])
                        E.tensor_tensor(
                            DM[:].rearrange("p (f a) -> p f a", f=nfx, a=L),
                            st["MUSv"], st["mup"], OP.subtract)
                        st["DM"] = DM
                        E.tensor_copy(st["augv"][:, :, :, 0:L], st["Sp4"])

                    # chain: RHS = A Sf -> aug cols 8:16
                    for st in streams:
                        mm_abt(st["E"], st["augv"][:, :, :, L:2 * L],
                               st["TMP"], st["At4"], st["Sf4"], st["nfx"])

                    # chain: GJ (width 16) -> jt4 = inv(Sp) (A Sf) = J^T
                    for st in streams:
                        gj(st["E"], st["augv"], st["PR"], st["T2"],
                           st["RC"], st["nfx"], 16)

                    # MUS' = muf + jt4^T DM
                    for st in streams:
                        nfx, E = st["nfx"], st["Eo"]
                        MUSn = pool.tile([128, nfx * L], F16,
                                         tag="MUSn" + st["name"],
                                         name="MUSn" + st["name"])
                        mus3 = MUSn[:].rearrange("p (f a) -> p f a",
                                                 f=nfx, a=L)
                        mv(E, mus3, st["MT"], st["augv"][:, :, :, L:2 * L],
                           st["DM"][:].rearrange("p (f a) -> p f a",
                                                 f=nfx, a=L),
                           nfx, kind="Atv")
                        E.tensor_tensor(mus3, st["muf"], mus3, OP.add)
                        st["MUSn"] = MUSn

                    # chain: Jc = jt4^T, T3 = Jc DS (ABt), SIGS' = T3 Jc^T + Sf
                    for st in streams:
                        nfx, E = st["nfx"], st["E"]
                        nm = st["name"]
                        jt4 = st["augv"][:, :, :, L:2 * L]
                        JC = pool.tile([128, nfx * LL], F16, tag="JC" + nm,
                                       name="JC" + nm)
                        E.tensor_copy(v4(JC[:], nfx),
                                      jt4.rearrange("p f k l -> p f l k"))
                        T3 = pool.tile([128, nfx * LL], F16, tag="T3" + nm,
                                       name="T3" + nm)
                        mm_abt(E, v4(T3[:], nfx), st["TMP"], v4(JC[:], nfx),
                               st["DS4"], nfx)
                        SIGSn = pool.tile([128, nfx * LL], F16,
                                          tag="SGn" + nm, name="SGn" + nm)
                        mm_abt(E, v4(SIGSn[:], nfx), st["TMP"],
                               v4(T3[:], nfx), v4(JC[:], nfx), nfx)
                        E.tensor_tensor(v4(SIGSn[:], nfx), v4(SIGSn[:], nfx),
                                        st["Sf4"], OP.add)
                        st["SIGSn"] = SIGSn

                    # pack fp32 output + store
                    if r >= W:
                        for st in streams:
                            nfx, off, E = st["nfx"], st["off"], st["Eo"]
                            PK = pool.tile([128, nfx * PW], F32,
                                           tag="PKo" + st["name"],
                                           name="PKo" + st["name"])
                            pko = PK[:].rearrange("p (f l j) -> p f l j",
                                                  f=nfx, l=L, j=L + 1)
                            E.tensor_copy(
                                pko[:, :, :, 0],
                                st["MUSn"][:].rearrange("p (f a) -> p f a",
                                                        f=nfx, a=L))
                            E.tensor_copy(pko[:, :, :, 1:L + 1],
                                          v4(st["SIGSn"][:], nfx))
                            tb0 = off * V + V + W - 1 - r
                            for h in range(2):
                                tb = h * TH + tb0
                                st["DQ"].dma_start(
                                    out.ap()[tb:tb + (nfx - 1) * V + 1:V,
                                             :, :, :]
                                    .rearrange("f b l j -> b f (l j)"),
                                    PK[h * NB:(h + 1) * NB, :]
                                    .rearrange("b (f e) -> b f e",
                                               f=nfx, e=PW))

                    for st in streams:
                        st["SIGSv"] = v4(st["SIGSn"][:], st["nfx"])
                        st["MUSv"] = st["MUSn"][:].rearrange(
                            "p (f a) -> p f a", f=st["nfx"], a=L)

    nc.compile()
    return nc


_CACHE = {}


def get_kernel(T=512, NFA=8, NFB=0, W=8):
    key = (T, NFA, NFB, W)
    if key not in _CACHE:
        _CACHE[key] = build_kernel(T=T, NFA=NFA, NFB=NFB, W=W)
    return _CACHE[key]


def make_in_maps(obs, A, C, mu_1, Sigma_1, Q, R, NFA=8):
    f32, f16 = np.float32, np.float16
    T = obs.shape[0]
    TH = T // 2
    cons = np.zeros((128, 2 * LL), f16)
    cons[:, :LL] = np.asarray(Q, f32).ravel()[None].astype(f16)
    cons[:, LL:] = np.eye(L, dtype=f32).ravel()[None].astype(f16)
    csig = np.zeros((128, NFA * LL), f16)
    cmu = np.zeros((128, NFA * L), f16)
    csig[0:NB, 0:LL] = (np.asarray(Sigma_1, f32)
                        - np.asarray(Q, f32)).ravel()[None].astype(f16)
    cmu[0:NB, 0:L] = np.asarray(mu_1, f32)[None].astype(f16)
    in_maps = []
    for c in range(NCORE):
        sl = slice(c * NB, (c + 1) * NB)
        Ac = np.asarray(A[sl], f32).reshape(NB, 2, TH, LL)
        Ac = np.ascontiguousarray(Ac.transpose(1, 0, 2, 3))
        Cc = np.asarray(C[sl], f32).reshape(NB, 2, TH, O * L)
        Cc = np.ascontiguousarray(Cc.transpose(1, 0, 2, 3))
        Yc = np.asarray(obs[:, sl], f32).reshape(2, TH, NB, O)
        Yc = np.ascontiguousarray(Yc.transpose(0, 2, 1, 3))
        in_maps.append({
            "a_in": Ac, "c_in": Cc, "y_in": Yc,
            "cons": cons, "csig": csig, "cmu": cmu,
        })
    return in_maps


def kalman_bass(obs, A, C, mu_1, Sigma_1, Q, R, T=512, nc=None):
    assert obs.shape[0] == T and obs.shape[1] == NB * NCORE
    if not np.allclose(R, np.eye(O), atol=1e-6):
        raise ValueError("general R not supported on device path")
    if nc is None:
        nc = get_kernel(T)
    in_maps = make_in_maps(obs, A, C, mu_1, Sigma_1, Q, R)
    res = bass_utils.run_bass_kernel_spmd(nc, in_maps,
                                          core_ids=list(range(NCORE)))
    return np.concatenate([res.results[c]["out"] for c in range(NCORE)],
                          axis=1)


# ---------------------------------------------------------------------------
# Slow numpy fallback (used only if the device path fails)
def _kalman_numpy(obs, A, C, mu_1, Sigma_1, Q, R):
    f32 = np.float32
    T, B, Oq = obs.shape
    Lq = mu_1.shape[0]
    At = np.ascontiguousarray(np.swapaxes(A, 0, 1)).astype(f32)
    Ct = np.ascontiguousarray(np.swapaxes(C, 0, 1)).astype(f32)
    I_L = np.eye(Lq, dtype=f32)

    def gj_solve(M, RHS):
        n = M.shape[1]
        Maug = np.concatenate([M, RHS], axis=-1).astype(f32)
        for p in range(n):
            recip = (f32(1.0) / Maug[:, p, p]).astype(f32)
            Maug[:, p, :] = Maug[:, p, :] * recip[:, None]
            col = Maug[:, :, p].copy()
            col[:, p] = 0.0
            Maug = (Maug - col[:, :, None] * Maug[:, p, None, :]).astype(f32)
        return Maug[:, :, n:]

    Rinv = np.linalg.inv(R.astype(np.float64)).astype(f32)
    use_R = not np.allclose(R, np.eye(Oq, dtype=f32))
    mu = np.broadcast_to(mu_1.astype(f32), (B, Lq)).copy()
    Sig = np.broadcast_to(Sigma_1.astype(f32), (B, Lq, Lq)).copy()
    mu_f = np.empty((T, B, Lq), f32)
    Sig_f = np.empty((T, B, Lq, Lq), f32)
    mu_p = np.empty((T, B, Lq), f32)
    Sig_p = np.empty((T, B, Lq, Lq), f32)
    for t in range(T):
        y, A_t, C_t = obs[t], At[t], Ct[t]
        mu_p[t] = mu
        Sig_p[t] = Sig
        Ceff = np.einsum('op,bpl->bol', Rinv, C_t) if use_R else C_t
        G = np.einsum('bol,bok->blk', C_t, Ceff)
        M = I_L[None] + np.matmul(Sig, G)
        r = y - np.einsum('bol,bl->bo', C_t, mu)
        ctr = np.einsum('bol,bo->bl', Ceff, r)
        b1 = np.einsum('blk,bk->bl', Sig, ctr)
        sol = gj_solve(M, np.concatenate([Sig, b1[:, :, None]], -1))
        Sig_z = np.ascontiguousarray(sol[:, :, :Lq])
        mu_z = mu + sol[:, :, Lq]
        mu_f[t] = mu_z
        Sig_f[t] = Sig_z
        mu = np.einsum('blk,bk->bl', A_t, mu_z).astype(f32)
        Sig = (np.matmul(np.matmul(A_t, Sig_z), np.swapaxes(A_t, 1, 2))
               + Q.astype(f32)).astype(f32)
    outp = np.empty((T, B, Lq, Lq + 1), f32)
    mu_s = mu_f[T - 1].copy()
    Sig_s = Sig_f[T - 1].copy()
    outp[T - 1, :, :, 0] = mu_s
    outp[T - 1, :, :, 1:] = Sig_s
    for t in range(T - 2, -1, -1):
        ASf = np.matmul(At[t + 1], Sig_f[t])
        Jt = gj_solve(Sig_p[t + 1], ASf)
        mu_s = (mu_f[t] + np.einsum('bkl,bk->bl', Jt, mu_s - mu_p[t + 1])).astype(f32)
        JdS = np.einsum('bkl,bkm->blm', Jt, Sig_s - Sig_p[t + 1])
        Sig_s = (Sig_f[t] + np.matmul(JdS, Jt)).astype(f32)
        outp[t, :, :, 0] = mu_s
        outp[t, :, :, 1:] = Sig_s
    return outp


def kernel(obs, A, C, mu_1, Sigma_1, Q, R):
    obs = np.asarray(obs, dtype=np.float32)
    A = np.asarray(A, dtype=np.float32)
    C = np.asarray(C, dtype=np.float32)
    mu_1 = np.asarray(mu_1, dtype=np.float32)
    Sigma_1 = np.asarray(Sigma_1, dtype=np.float32)
    Q = np.asarray(Q, dtype=np.float32)
    R = np.asarray(R, dtype=np.float32)
    try:
        return kalman_bass(obs, A, C, mu_1, Sigma_1, Q, R)
    except Exception:
        import traceback
        traceback.print_exc()
        return _kalman_numpy(obs, A, C, mu_1, Sigma_1, Q, R)


# revision 17
# speedup vs baseline: 1.1449x; 1.1449x over previous
"""Kalman filter + RTS smoother kernel for Trainium2 (8 NeuronCores).

T=512 steps, B=512 batch, L=8 latent, O=16 obs. Data-parallel over B
(64/core); on each core the T axis is split into 16 chunks (2 on the
partition axis x 8 on the free axis, V=32 steps each) that run
concurrently, each with a W-step discarded warmup exploiting the
exponential forgetting (~0.4/step) of the Kalman recursions.

The 8 free-axis chunks are split into two independent instruction
streams: stream A's covariance chain runs on the Vector engine (DVE,
fp16 2x fast path), stream B's on the GpSimd/Pool engine, so the two
serial recursions pipeline against each other instead of serializing
on one engine. All 8x8 matmuls are emitted in ABt broadcast form
(unit innermost strides on both operands, exploiting the symmetry of
the covariance operands) with add-fold trees instead of TensorReduce.
The filter solve is a batched no-pivot Gauss-Jordan on the augmented
system [I + Sig G | Sig | Sig v] (divide ALU op, no reciprocal), which
yields Sig_z and the mu update in one elimination. All staging DRAM
(G/A/cy and the smoother inputs) is fp16 in h-major planes so every
per-step load/store is a single DMA.
"""
import sys
import numpy as np

for _p in ("/opt/trn_rl_repo",):
    if _p not in sys.path:
        sys.path.append(_p)

import concourse.bass as bass
import concourse.bacc as bacc
import concourse.mybir as mybir
from concourse.tile import TileContext
from concourse import bass_utils

F32 = mybir.dt.float32
F16 = mybir.dt.float16
AX = mybir.AxisListType
OP = mybir.AluOpType

L = 8
O = 16
LL = L * L
NB = 64
NCORE = 8
FW = 2 * LL + L          # fwd staging row: [G | A | cy] = 136
SW = 2 * (LL + L)        # smoother staging row: [Sf | muf | Sp(t+1) | mup(t+1)] = 144
PW = L * (L + 1)         # packed output row = 72


def build_kernel(T=512, NFA=8, NFB=0, W=7, TB=16,
                 pack_a="pool", pack_b="dve"):
    NF = NFA + NFB
    TH = T // 2
    V = TH // NF
    assert NF * V == TH and V >= W
    NSTEP = V + W
    ULEN = TH + 2 * W + 1
    SLEN = TH + 2 * W + 1

    nc = bacc.Bacc("TRN2", target_bir_lowering=False, debug=False,
                   num_devices=NCORE)

    a_in = nc.dram_tensor("a_in", (2, NB, TH, LL), F32, kind="ExternalInput")
    c_in = nc.dram_tensor("c_in", (2, NB, TH, O * L), F32, kind="ExternalInput")
    y_in = nc.dram_tensor("y_in", (2, NB, TH, O), F32, kind="ExternalInput")
    cons = nc.dram_tensor("cons", (128, 2 * LL), F16, kind="ExternalInput")
    csig = nc.dram_tensor("csig", (128, NFA * LL), F16, kind="ExternalInput")
    cmu = nc.dram_tensor("cmu", (128, NFA * L), F16, kind="ExternalInput")
    out = nc.dram_tensor("out", (T, NB, L, L + 1), F32, kind="ExternalOutput")

    FWD = nc.dram_tensor("fwd", (2, NB, ULEN, FW), F16, kind="Internal")
    SPFM = nc.dram_tensor("spfm", (2, NB, SLEN, SW), F16, kind="Internal")
    fwdv = FWD.ap().rearrange("h b u e -> (h b) u e")
    spfmv = SPFM.ap().rearrange("h b u e -> (h b) u e")

    VE = None  # set inside context

    def v4(flat_ap, nfx):
        return flat_ap.rearrange("p (f i k) -> p f i k", f=nfx, i=L, k=L)

    with TileContext(nc) as tc:
        with tc.tile_pool(name="const", bufs=1) as cpool:
            VE, PO = nc.vector, nc.gpsimd

            def eng(name):
                return VE if name == "dve" else PO

            # ---------------- constants ----------------
            CT = cpool.tile([128, 2 * LL], F16)
            nc.sync.dma_start(CT[:], cons.ap())
            Qv = CT[:, 0:LL].rearrange("p (i k) -> p i k", i=L, k=L)
            Iv = CT[:, LL:2 * LL].rearrange("p (i k) -> p i k", i=L, k=L)
            CS = cpool.tile([128, NFA * LL], F16)
            CM = cpool.tile([128, NFA * L], F16)
            nc.sync.dma_start(CS[:], csig.ap())
            nc.sync.dma_start(CM[:], cmu.ap())

            # ---------------- DRAM pads ----------------
            # FWD h=0 plane rows [0, W): zeros (chunk-0 warmup reads them)
            ZP = cpool.tile([128, W * FW], F16)
            VE.memset(ZP[:], 0.0)
            nc.sync.dma_start(
                FWD.ap()[0, :, 0:W, :],
                ZP[0:NB, :].rearrange("b (u e) -> b u e", u=W, e=FW))
            # FWD both planes row range [TH+W, TH+2W]: zero A pad for h=1
            # (t >= T => J=0 boundary); h=0 range is overwritten by the
            # dram-dram dup below.
            ZP2 = cpool.tile([128, (W + 1) * FW], F16)
            VE.memset(ZP2[:], 0.0)
            for h in range(2):
                nc.sync.dma_start(
                    FWD.ap()[h, :, TH + W:TH + 2 * W + 1, :],
                    ZP2[0:NB, :].rearrange("b (u e) -> b u e", u=W + 1, e=FW))
            # SPFM h=1 tail pad rows [TH+W, TH+2W]: [I | 0 | I | 0]
            SPAD = cpool.tile([128, (W + 1) * SW], F16)
            VE.memset(SPAD[:], 0.0)
            spadv = SPAD[:].rearrange("p (u e) -> p u e", u=W + 1, e=SW)
            for j in range(W + 1):
                PO.tensor_copy(spadv[:, j, 0:LL]
                               .rearrange("p (i k) -> p i k", i=L, k=L), Iv)
                PO.tensor_copy(spadv[:, j, LL + L:2 * LL + L]
                               .rearrange("p (i k) -> p i k", i=L, k=L), Iv)
            nc.sync.dma_start(
                SPFM.ap()[1, :, TH + W:TH + 2 * W + 1, :],
                SPAD[64:128, :].rearrange("b (u e) -> b u e", u=W + 1, e=SW))

            # ---------------- prep: G = C^T C, cy = C^T y, stage A ----------
            with tc.tile_pool(name="prep", bufs=2) as ppool:
                NIT = TH // TB
                for j in range(NIT):
                    E = VE if (j % 4) != 3 else PO
                    DQ = nc.sync
                    At = ppool.tile([128, TB * LL], F32, tag="pA")
                    Cr = ppool.tile([128, TB * O * L], F32, tag="pC")
                    Yr = ppool.tile([128, TB * O], F32, tag="pY")
                    DQ.dma_start(
                        At[:].rearrange("p (u e) -> p u e", u=TB, e=LL),
                        a_in.ap().rearrange("h b u e -> (h b) u e")
                        [:, j * TB:(j + 1) * TB, :])
                    DQ.dma_start(
                        Cr[:].rearrange("p (u e) -> p u e", u=TB, e=O * L),
                        c_in.ap().rearrange("h b u e -> (h b) u e")
                        [:, j * TB:(j + 1) * TB, :])
                    DQ.dma_start(
                        Yr[:].rearrange("p (u e) -> p u e", u=TB, e=O),
                        y_in.ap().rearrange("h b u e -> (h b) u e")
                        [:, j * TB:(j + 1) * TB, :])
                    PKF = ppool.tile([128, TB * FW], F16, tag="pF")
                    pkv = PKF[:].rearrange("p (u e) -> p u e", u=TB, e=FW)
                    # cast A into pack
                    E.tensor_copy(pkv[:, :, LL:2 * LL],
                                  At[:].rearrange("p (u e) -> p u e",
                                                  u=TB, e=LL))
                    # C^T (transposed cast): (u, l, o) <- (u, o, l)
                    CtT = ppool.tile([128, TB * O * L], F16, tag="pCt")
                    ctv = CtT[:].rearrange("p (u l o) -> p u l o",
                                           u=TB, l=L, o=O)
                    E.tensor_copy(ctv,
                                  Cr[:].rearrange("p (u o l) -> p u o l",
                                                  u=TB, o=O, l=L)
                                  .rearrange("p u o l -> p u l o"))
                    Y16 = ppool.tile([128, TB * O], F16, tag="pY16")
                    E.tensor_copy(Y16[:], Yr[:])
                    # G mults: per u, (l, m, o) = Ct[l,o] * Ct[m,o]
                    GT = ppool.tile([128, TB * L * L * O], F16, tag="pG")
                    gtv = GT[:].rearrange("p (u l m o) -> p u l m o",
                                          u=TB, l=L, m=L, o=O)
                    for u in range(TB):
                        E.tensor_tensor(
                            gtv[:, u],
                            ctv[:, u].unsqueeze(2).broadcast_to((128, L, L, O)),
                            ctv[:, u].unsqueeze(1).broadcast_to((128, L, L, O)),
                            OP.mult)
                    gk = GT[:].rearrange("p (x o) -> p x o", o=O)
                    E.tensor_tensor(gk[:, :, 0:8], gk[:, :, 0:8],
                                    gk[:, :, 8:16], OP.add)
                    E.tensor_tensor(gk[:, :, 0:4], gk[:, :, 0:4],
                                    gk[:, :, 4:8], OP.add)
                    E.tensor_tensor(gk[:, :, 0:2], gk[:, :, 0:2],
                                    gk[:, :, 2:4], OP.add)
                    gkv = GT[:].rearrange("p (u x o) -> p u x o",
                                          u=TB, x=LL, o=O)
                    E.tensor_tensor(pkv[:, :, 0:LL],
                                    gkv[:, :, :, 0], gkv[:, :, :, 1], OP.add)
                    # cy: (u, l, o) = Ct[l,o] * y[o]
                    CYT = ppool.tile([128, TB * L * O], F16, tag="pcy")
                    cyv = CYT[:].rearrange("p (u l o) -> p u l o",
                                           u=TB, l=L, o=O)
                    E.tensor_tensor(
                        cyv, ctv,
                        Y16[:].rearrange("p (u o) -> p u o", u=TB, o=O)
                        .unsqueeze(2).broadcast_to((128, TB, L, O)),
                        OP.mult)
                    ck = CYT[:].rearrange("p (x o) -> p x o", o=O)
                    E.tensor_tensor(ck[:, :, 0:8], ck[:, :, 0:8],
                                    ck[:, :, 8:16], OP.add)
                    E.tensor_tensor(ck[:, :, 0:4], ck[:, :, 0:4],
                                    ck[:, :, 4:8], OP.add)
                    E.tensor_tensor(ck[:, :, 0:2], ck[:, :, 0:2],
                                    ck[:, :, 2:4], OP.add)
                    ckv = CYT[:].rearrange("p (u l o) -> p u l o",
                                           u=TB, l=L, o=O)
                    E.tensor_tensor(pkv[:, :, 2 * LL:2 * LL + L],
                                    ckv[:, :, :, 0], ckv[:, :, :, 1], OP.add)
                    DQ.dma_start(
                        fwdv[:, W + j * TB:W + (j + 1) * TB, :], pkv)

            # dram-dram dups across the h boundary:
            # FWD h=1 front pad [0, W) <- h=0 rows [TH, TH+W)
            nc.sync.dma_start(FWD.ap()[1, :, 0:W, :],
                              FWD.ap()[0, :, TH:TH + W, :])
            # FWD h=0 tail rows [TH+W, TH+2W] <- h=1 rows [W, 2W]
            nc.sync.dma_start(FWD.ap()[0, :, TH + W:TH + 2 * W + 1, :],
                              FWD.ap()[1, :, W:2 * W + 1, :])

            # ---------------- helper emitters ----------------
            def mm_abt(E, dstv, tmp, x4, y4, nfx):
                """dst[f,a,c] = sum_k x4[f,a,k] * y4[f,c,k]."""
                tv = tmp[:].rearrange("p (f a c k) -> p f a c k",
                                      f=nfx, a=L, c=L, k=L)
                for i in range(nfx):
                    E.tensor_tensor(
                        tv[:, i],
                        x4[:, i].unsqueeze(2).broadcast_to((128, L, L, L)),
                        y4[:, i].unsqueeze(1).broadcast_to((128, L, L, L)),
                        OP.mult)
                tk = tmp[:, 0:nfx * 512].rearrange("p (x k) -> p x k", k=L)
                E.tensor_tensor(tk[:, :, 0:4], tk[:, :, 0:4],
                                tk[:, :, 4:8], OP.add)
                E.tensor_tensor(tk[:, :, 0:2], tk[:, :, 0:2],
                                tk[:, :, 2:4], OP.add)
                E.tensor_tensor(
                    dstv,
                    tk[:, :, 0].rearrange("p (f a c) -> p f a c",
                                          f=nfx, a=L, c=L),
                    tk[:, :, 1].rearrange("p (f a c) -> p f a c",
                                          f=nfx, a=L, c=L),
                    OP.add)

            def mv(E, dst3, tmp, x4, v3, nfx, kind="Av"):
                """dst[f,a] = sum_k X[f,a,k] v[f,k] (Av) or X[f,k,a] v[f,k].
                v3 is a (p, f, k) view."""
                xv = x4 if kind == "Av" else x4.rearrange("p f k l -> p f l k")
                vv = v3.unsqueeze(2).broadcast_to((128, nfx, L, L))
                tv = tmp[:, 0:nfx * LL].rearrange("p (f a k) -> p f a k",
                                                  f=nfx, a=L, k=L)
                E.tensor_tensor(tv, xv, vv, OP.mult)
                E.tensor_tensor(tv[:, :, :, 0:4], tv[:, :, :, 0:4],
                                tv[:, :, :, 4:8], OP.add)
                E.tensor_tensor(tv[:, :, :, 0:2], tv[:, :, :, 0:2],
                                tv[:, :, :, 2:4], OP.add)
                E.tensor_tensor(dst3, tv[:, :, :, 0], tv[:, :, :, 1], OP.add)

            def gj(E, augv, prt, t2t, crt, rct, nfx, wid):
                prtv = prt[:].rearrange("p (r f j) -> p r f j",
                                        r=2, f=nfx, j=wid - 1)
                rctv = rct[:].rearrange("p (r f j) -> p r f j",
                                        r=2, f=nfx, j=1)
                pend = None
                for pp in range(L):
                    jw = wid - 1 - pp
                    prv = prtv[:, pp % 2][:, :, 0:jw]
                    rcv = rctv[:, pp % 2]
                    with nc.allow_low_precision(
                            reason="fp16 pivot reciprocal, pivots are O(1)"):
                        VE.reciprocal(rcv, augv[:, :, pp, pp:pp + 1])
                    E.tensor_tensor(
                        prv, augv[:, :, pp, pp + 1:],
                        rcv.broadcast_to((128, nfx, jw)), OP.mult)
                    if pend is not None:
                        E.tensor_copy(augv[:, :, pend[0], pend[0] + 1:],
                                      pend[1])
                    crv = crt[:].rearrange("p (f i j) -> p f i j",
                                           f=nfx, i=L, j=wid - 1)[:, :, :, 0:jw]
                    E.tensor_copy(crv,
                                  augv[:, :, :, pp:pp + 1]
                                  .broadcast_to((128, nfx, L, jw)))
                    t2v = t2t[:].rearrange("p (f i j) -> p f i j",
                                           f=nfx, i=L, j=wid - 1)[:, :, :, 0:jw]
                    E.tensor_tensor(
                        t2v, crv,
                        prv.unsqueeze(2).broadcast_to((128, nfx, L, jw)),
                        OP.mult)
                    E.tensor_tensor(augv[:, :, :, pp + 1:],
                                    augv[:, :, :, pp + 1:], t2v, OP.subtract)
                    pend = (pp, prv)
                E.tensor_copy(augv[:, :, L - 1, L:], pend[1])

            # ---------------- stream descriptors ----------------
            # Each stream is fully self-contained on one compute engine and
            # one DMA-issue queue (ACT / PE sequencers are otherwise idle),
            # so the two serial recursions never cross-stall. Loads run two
            # steps ahead of the recursion.
            streams = [
                dict(nfx=NFA, off=0, E=VE, Eo=PO, name="a", DQ=nc.sync),
            ]

            with tc.tile_pool(name="work", bufs=3) as pool, \
                 tc.tile_pool(name="work2", bufs=2) as pool2:

                def ft_load(st, s):
                    nfx, off = st["nfx"], st["off"]
                    FT = pool.tile([128, nfx * FW], F16,
                                   tag="FT" + st["name"],
                                   name="FT" + st["name"], bufs=4)
                    st["DQ"].dma_start(
                        FT[:].rearrange("p (f e) -> p f e", f=nfx, e=FW),
                        fwdv[:, off * V + s:
                             off * V + s + (nfx - 1) * V + 1:V, :])
                    st["ftq"].append(FT)

                # ============ FORWARD FILTER ============
                for st in streams:
                    nfx = st["nfx"]
                    st["SIG"] = cpool.tile([128, nfx * LL], F16,
                                           name="SIG0" + st["name"])
                    st["MU"] = cpool.tile([128, nfx * L], F16,
                                          name="MU0" + st["name"])
                    st["E"].memset(st["SIG"][:], 0.0)
                    st["E"].memset(st["MU"][:], 0.0)
                    st["ftq"] = []
                    ft_load(st, 0)
                    ft_load(st, 1)

                for s in range(NSTEP):
                    for st in streams:
                        nfx = st["nfx"]
                        if s + 2 < NSTEP:
                            ft_load(st, s + 2)
                        FT = st["ftq"].pop(0)
                        ftv = FT[:].rearrange("p (f e) -> p f e", f=nfx, e=FW)
                        st["Gt4"] = ftv[:, :, 0:LL].rearrange(
                            "p f (i k) -> p f i k", i=L, k=L)
                        st["At4"] = ftv[:, :, LL:2 * LL].rearrange(
                            "p f (i k) -> p f i k", i=L, k=L)
                        st["CYf"] = ftv[:, :, 2 * LL:2 * LL + L]

                    if s == W:
                        stA = streams[0]
                        VE.tensor_tensor(stA["SIG"][:], stA["SIG"][:], CS[:],
                                         OP.add)
                        VE.tensor_tensor(stA["MU"][:], stA["MU"][:], CM[:],
                                         OP.add)

                    for st in streams:
                        nfx = st["nfx"]
                        nm = st["name"]
                        st["AUG"] = pool2.tile([128, nfx * L * 17], F16,
                                               tag="AUG" + nm, name="AUG" + nm)
                        st["augv"] = st["AUG"][:].rearrange(
                            "p (f i j) -> p f i j", f=nfx, i=L, j=17)
                        st["TMP"] = pool2.tile([128, nfx * 512], F16,
                                               tag="TMP" + nm, name="TMP" + nm)
                        st["PR"] = pool.tile([128, nfx * 2 * 16], F16,
                                             tag="PR" + nm, name="PR" + nm)
                        st["T2"] = pool2.tile([128, nfx * L * 16], F16,
                                              tag="T2" + nm, name="T2" + nm)
                        st["MT"] = pool.tile([128, nfx * LL], F16,
                                             tag="MT" + nm, name="MT" + nm)
                        st["RC"] = pool.tile([128, nfx * 2], F16,
                                             tag="RC" + nm, name="RC" + nm)
                        st["CR"] = pool.tile([128, nfx * L * 16], F16,
                                             tag="CR" + nm, name="CR" + nm)

                    # aug assembly: cols 8:16 = Sig, col 16 = Sig cy + mu
                    # (mu_z = M^-1 (Sig cy + mu) -- push-through identity)
                    for st in streams:
                        nfx, Eo = st["nfx"], st["Eo"]
                        sig4 = v4(st["SIG"][:], nfx)
                        Eo.tensor_copy(st["augv"][:, :, :, L:2 * L], sig4)
                        mv(Eo, st["augv"][:, :, :, 16], st["MT"], sig4,
                           st["CYf"], nfx)
                        Eo.tensor_tensor(
                            st["augv"][:, :, :, 16],
                            st["augv"][:, :, :, 16],
                            st["MU"][:].rearrange("p (f a) -> p f a",
                                                  f=nfx, a=L), OP.add)

                    # chain: SG mm -> aug cols 0:8, then +I
                    for st in streams:
                        nfx, E = st["nfx"], st["E"]
                        mm_abt(E, st["augv"][:, :, :, 0:L], st["TMP"],
                               v4(st["SIG"][:], nfx), st["Gt4"], nfx)
                        diag = st["AUG"][:].rearrange(
                            "p (f e) -> p f e", f=nfx, e=L * 17)[:, :, 0:L * 17 - 1:18]
                        E.tensor_scalar(diag, diag, 1.0, None, OP.add)

                    # chain: GJ -> [I | Sig_z | mu_z]
                    for st in streams:
                        gj(st["E"], st["augv"], st["PR"], st["T2"],
                           st["CR"], st["RC"], st["nfx"], 17)

                    # mu' = A mu_z
                    for st in streams:
                        nfx, Eo = st["nfx"], st["Eo"]
                        MUn = pool.tile([128, nfx * L], F16,
                                        tag="MUn" + st["name"],
                                        name="MUn" + st["name"])
                        mv(Eo, MUn[:].rearrange("p (f a) -> p f a",
                                                f=nfx, a=L),
                           st["MT"], st["At4"], st["augv"][:, :, :, 16], nfx)
                        st["MUn"] = MUn

                    # chain: ASZ = A Sigz, SIG' = ASZ A^T + Q
                    for st in streams:
                        nfx, E = st["nfx"], st["E"]
                        sigz4 = st["augv"][:, :, :, L:2 * L]
                        ASZ = pool.tile([128, nfx * LL], F16,
                                        tag="ASZ" + st["name"],
                                        name="ASZ" + st["name"])
                        mm_abt(E, v4(ASZ[:], nfx), st["TMP"], st["At4"],
                               sigz4, nfx)
                        SIGn = pool.tile([128, nfx * LL], F16,
                                         tag="SIGn" + st["name"],
                                         name="SIGn" + st["name"])
                        mm_abt(E, v4(SIGn[:], nfx), st["TMP"],
                               v4(ASZ[:], nfx), st["At4"], nfx)
                        E.tensor_tensor(
                            v4(SIGn[:], nfx), v4(SIGn[:], nfx),
                            Qv.unsqueeze(1).broadcast_to((128, nfx, L, L)),
                            OP.add)
                        st["SIGn"] = SIGn

                    # pack [Sf | muf | Sp(t+1) | mup(t+1)] and store
                    if s >= W:
                        for st in streams:
                            nfx, off, E = st["nfx"], st["off"], st["Eo"]
                            PACK = pool.tile([128, nfx * SW], F16,
                                             tag="PK" + st["name"],
                                             name="PK" + st["name"])
                            pkv = PACK[:].rearrange("p (f e) -> p f e",
                                                    f=nfx, e=SW)
                            E.tensor_copy(
                                pkv[:, :, 0:LL].rearrange(
                                    "p f (i k) -> p f i k", i=L, k=L),
                                st["augv"][:, :, :, L:2 * L])
                            E.tensor_copy(pkv[:, :, LL:LL + L],
                                          st["augv"][:, :, :, 16])
                            E.tensor_copy(
                                pkv[:, :, LL + L:2 * LL + L].rearrange(
                                    "p f (i k) -> p f i k", i=L, k=L),
                                v4(st["SIGn"][:], nfx))
                            E.tensor_copy(
                                pkv[:, :, 2 * LL + L:SW],
                                st["MUn"][:].rearrange("p (f a) -> p f a",
                                                       f=nfx, a=L))
                            st["DQ"].dma_start(
                                spfmv[:, off * V + s:
                                      off * V + s + (nfx - 1) * V + 1:V, :],
                                pkv)
                            if st["off"] == 0 and W <= s <= 2 * W:
                                st["DQ"].dma_start(
                                    SPFM.ap()[0, :, TH + s, :],
                                    PACK[64:128, 0:SW])

                    for st in streams:
                        st["SIG"] = st["SIGn"]
                        st["MU"] = st["MUn"]

                # ============ BACKWARD SMOOTHER ============
                def bwd_load(st, r):
                    nfx, off, nm = st["nfx"], st["off"], st["name"]
                    SFT = pool.tile([128, nfx * SW], F16, tag="SF" + nm,
                                    name="SF" + nm, bufs=4)
                    row = off * V + V + 2 * W - 1 - r
                    st["DQ"].dma_start(
                        SFT[:].rearrange("p (f e) -> p f e", f=nfx, e=SW),
                        spfmv[:, row:row + (nfx - 1) * V + 1:V, :])
                    At1 = pool.tile([128, nfx * LL], F16, tag="Ab" + nm,
                                    name="Ab" + nm, bufs=4)
                    st["DQ"].dma_start(
                        At1[:].rearrange("p (f e) -> p f e", f=nfx, e=LL),
                        fwdv[:, row + 1:row + 1 + (nfx - 1) * V + 1:V,
                             LL:2 * LL])
                    st["bq"].append((SFT, At1))

                for st in streams:
                    nfx, off = st["nfx"], st["off"]
                    INIT = cpool.tile([128, nfx * (LL + L)], F16,
                                      name="INIT" + st["name"])
                    st["DQ"].dma_start(
                        INIT[:].rearrange("p (f e) -> p f e",
                                          f=nfx, e=LL + L),
                        spfmv[:, off * V + V + 2 * W:
                              off * V + V + 2 * W + (nfx - 1) * V + 1:V,
                              0:LL + L])
                    iv = INIT[:].rearrange("p (f e) -> p f e", f=nfx, e=LL + L)
                    st["SIGSv"] = iv[:, :, 0:LL].rearrange(
                        "p f (i k) -> p f i k", i=L, k=L)
                    st["MUSv"] = iv[:, :, LL:LL + L]
                    st["bq"] = []
                    bwd_load(st, 0)
                    bwd_load(st, 1)

                for r in range(NSTEP):
                    for st in streams:
                        nfx = st["nfx"]
                        nm = st["name"]
                        if r + 2 < NSTEP:
                            bwd_load(st, r + 2)
                        SFT, At1 = st["bq"].pop(0)
                        sfv = SFT[:].rearrange("p (f e) -> p f e", f=nfx, e=SW)
                        st["Sf4"] = sfv[:, :, 0:LL].rearrange(
                            "p f (i k) -> p f i k", i=L, k=L)
                        st["muf"] = sfv[:, :, LL:LL + L]
                        st["Sp4"] = sfv[:, :, LL + L:2 * LL + L].rearrange(
                            "p f (i k) -> p f i k", i=L, k=L)
                        st["mup"] = sfv[:, :, 2 * LL + L:SW]
                        st["At4"] = v4(At1[:], nfx)

                        st["AUG"] = pool2.tile([128, nfx * L * 16], F16,
                                               tag="AUG" + nm,
                                               name="AUGb" + nm)
                        st["augv"] = st["AUG"][:].rearrange(
                            "p (f i j) -> p f i j", f=nfx, i=L, j=16)
                        st["TMP"] = pool2.tile([128, nfx * 512], F16,
                                               tag="TMP" + nm,
                                               name="TMPb" + nm)
                        st["PR"] = pool.tile([128, nfx * 2 * 15], F16,
                                             tag="PR" + nm, name="PRb" + nm)
                        st["T2"] = pool2.tile([128, nfx * L * 15], F16,
                                              tag="T2" + nm, name="T2b" + nm)
                        st["MT"] = pool.tile([128, nfx * LL], F16,
                                             tag="MT" + nm, name="MTb" + nm)
                        st["RC"] = pool.tile([128, nfx * 2], F16,
                                             tag="RC" + nm, name="RCb" + nm)
                        st["CR"] = pool.tile([128, nfx * L * 15], F16,
                                             tag="CR" + nm, name="CRb" + nm)

                    # DS, DM, aug Sp copy
                    for st in streams:
                        nfx, E = st["nfx"], st["Eo"]
                        DS = pool.tile([128, nfx * LL], F16,
                                       tag="DS" + st["name"],
                                       name="DS" + st["name"])
                        E.tensor_tensor(v4(DS[:], nfx), st["SIGSv"],
                                        st["Sp4"], OP.subtract)
                        st["DS4"] = v4(DS[:], nfx)
                        DM = pool.tile([128, nfx * L], F16,
                                       tag="DM" + st["name"],
                                       name="DM" + st["name"])
                        E.tensor_tensor(
                            DM[:].rearrange("p (f a) -> p f a", f=nfx, a=L),
                            st["MUSv"], st["mup"], OP.subtract)
                        st["DM"] = DM
                        E.tensor_copy(st["augv"][:, :, :, 0:L], st["Sp4"])

                    # chain: RHS = A Sf -> aug cols 8:16
                    for st in streams:
                        mm_abt(st["E"], st["augv"][:, :, :, L:2 * L],
                               st["TMP"], st["At4"], st["Sf4"], st["nfx"])

                    # chain: GJ (width 16) -> jt4 = inv(Sp) (A Sf) = J^T
                    for st in streams:
                        gj(st["E"], st["augv"], st["PR"], st["T2"],
                           st["CR"], st["RC"], st["nfx"], 16)

                    # MUS' = muf + jt4^T DM
                    for st in streams:
                        nfx, E = st["nfx"], st["Eo"]
                        MUSn = pool.tile([128, nfx * L], F16,
                                         tag="MUSn" + st["name"],
                                         name="MUSn" + st["name"])
                        mus3 = MUSn[:].rearrange("p (f a) -> p f a",
                                                 f=nfx, a=L)
                        mv(E, mus3, st["MT"], st["augv"][:, :, :, L:2 * L],
                           st["DM"][:].rearrange("p (f a) -> p f a",
                                                 f=nfx, a=L),
                           nfx, kind="Atv")
                        E.tensor_tensor(mus3, st["muf"], mus3, OP.add)
                        st["MUSn"] = MUSn

                    # chain: Jc = jt4^T, T3 = Jc DS (ABt), SIGS' = T3 Jc^T + Sf
                    for st in streams:
                        nfx, E = st["nfx"], st["E"]
                        nm = st["name"]
                        jt4 = st["augv"][:, :, :, L:2 * L]
                        JC = pool.tile([128, nfx * LL], F16, tag="JC" + nm,
                                       name="JC" + nm)
                        E.tensor_copy(v4(JC[:], nfx),
                                      jt4.rearrange("p f k l -> p f l k"))
                        T3 = pool.tile([128, nfx * LL], F16, tag="T3" + nm,
                                       name="T3" + nm)
                        mm_abt(E, v4(T3[:], nfx), st["TMP"], v4(JC[:], nfx),
                               st["DS4"], nfx)
                        SIGSn = pool.tile([128, nfx * LL], F16,
                                          tag="SGn" + nm, name="SGn" + nm)
                        mm_abt(E, v4(SIGSn[:], nfx), st["TMP"],
                               v4(T3[:], nfx), v4(JC[:], nfx), nfx)
                        E.tensor_tensor(v4(SIGSn[:], nfx), v4(SIGSn[:], nfx),
                                        st["Sf4"], OP.add)
                        st["SIGSn"] = SIGSn

                    # pack fp32 output + store
                    if r >= W:
                        for st in streams:
                            nfx, off, E = st["nfx"], st["off"], st["Eo"]
                            PK = pool.tile([128, nfx * PW], F32,
                                           tag="PKo" + st["name"],
                                           name="PKo" + st["name"])
                            pko = PK[:].rearrange("p (f l j) -> p f l j",
                                                  f=nfx, l=L, j=L + 1)
                            E.tensor_copy(
                                pko[:, :, :, 0],
                                st["MUSn"][:].rearrange("p (f a) -> p f a",
                                                        f=nfx, a=L))
                            E.tensor_copy(pko[:, :, :, 1:L + 1],
                                          v4(st["SIGSn"][:], nfx))
                            tb0 = off * V + V + W - 1 - r
                            for h in range(2):
                                tb = h * TH + tb0
                                st["DQ"].dma_start(
                                    out.ap()[tb:tb + (nfx - 1) * V + 1:V,
                                             :, :, :]
                                    .rearrange("f b l j -> b f (l j)"),
                                    PK[h * NB:(h + 1) * NB, :]
                                    .rearrange("b (f e) -> b f e",
                                               f=nfx, e=PW))

                    for st in streams:
                        st["SIGSv"] = v4(st["SIGSn"][:], st["nfx"])
                        st["MUSv"] = st["MUSn"][:].rearrange(
                            "p (f a) -> p f a", f=st["nfx"], a=L)

    nc.compile()
    return nc


_CACHE = {}


def get_kernel(T=512, NFA=8, NFB=0, W=7):
    key = (T, NFA, NFB, W)
    if key not in _CACHE:
        _CACHE[key] = build_kernel(T=T, NFA=NFA, NFB=NFB, W=W)
    return _CACHE[key]


def make_in_maps(obs, A, C, mu_1, Sigma_1, Q, R, NFA=8):
    f32, f16 = np.float32, np.float16
    T = obs.shape[0]
    TH = T // 2
    cons = np.zeros((128, 2 * LL), f16)
    cons[:, :LL] = np.asarray(Q, f32).ravel()[None].astype(f16)
    cons[:, LL:] = np.eye(L, dtype=f32).ravel()[None].astype(f16)
    csig = np.zeros((128, NFA * LL), f16)
    cmu = np.zeros((128, NFA * L), f16)
    csig[0:NB, 0:LL] = (np.asarray(Sigma_1, f32)
                        - np.asarray(Q, f32)).ravel()[None].astype(f16)
    cmu[0:NB, 0:L] = np.asarray(mu_1, f32)[None].astype(f16)
    in_maps = []
    for c in range(NCORE):
        sl = slice(c * NB, (c + 1) * NB)
        Ac = np.asarray(A[sl], f32).reshape(NB, 2, TH, LL)
        Ac = np.ascontiguousarray(Ac.transpose(1, 0, 2, 3))
        Cc = np.asarray(C[sl], f32).reshape(NB, 2, TH, O * L)
        Cc = np.ascontiguousarray(Cc.transpose(1, 0, 2, 3))
        Yc = np.asarray(obs[:, sl], f32).reshape(2, TH, NB, O)
        Yc = np.ascontiguousarray(Yc.transpose(0, 2, 1, 3))
        in_maps.append({
            "a_in": Ac, "c_in": Cc, "y_in": Yc,
            "cons": cons, "csig": csig, "cmu": cmu,
        })
    return in_maps


def kalman_bass(obs, A, C, mu_1, Sigma_1, Q, R, T=512, nc=None):
    assert obs.shape[0] == T and obs.shape[1] == NB * NCORE
    if not np.allclose(R, np.eye(O), atol=1e-6):
        raise ValueError("general R not supported on device path")
    if nc is None:
        nc = get_kernel(T)
    in_maps = make_in_maps(obs, A, C, mu_1, Sigma_1, Q, R)
    res = bass_utils.run_bass_kernel_spmd(nc, in_maps,
                                          core_ids=list(range(NCORE)))
    return np.concatenate([res.results[c]["out"] for c in range(NCORE)],
                          axis=1)


# ---------------------------------------------------------------------------
# Slow numpy fallback (used only if the device path fails)
def _kalman_numpy(obs, A, C, mu_1, Sigma_1, Q, R):
    f32 = np.float32
    T, B, Oq = obs.shape
    Lq = mu_1.shape[0]
    At = np.ascontiguousarray(np.swapaxes(A, 0, 1)).astype(f32)
    Ct = np.ascontiguousarray(np.swapaxes(C, 0, 1)).astype(f32)
    I_L = np.eye(Lq, dtype=f32)

    def gj_solve(M, RHS):
        n = M.shape[1]
        Maug = np.concatenate([M, RHS], axis=-1).astype(f32)
        for p in range(n):
            recip = (f32(1.0) / Maug[:, p, p]).astype(f32)
            Maug[:, p, :] = Maug[:, p, :] * recip[:, None]
            col = Maug[:, :, p].copy()
            col[:, p] = 0.0
            Maug = (Maug - col[:, :, None] * Maug[:, p, None, :]).astype(f32)
        return Maug[:, :, n:]

    Rinv = np.linalg.inv(R.astype(np.float64)).astype(f32)
    use_R = not np.allclose(R, np.eye(Oq, dtype=f32))
    mu = np.broadcast_to(mu_1.astype(f32), (B, Lq)).copy()
    Sig = np.broadcast_to(Sigma_1.astype(f32), (B, Lq, Lq)).copy()
    mu_f = np.empty((T, B, Lq), f32)
    Sig_f = np.empty((T, B, Lq, Lq), f32)
    mu_p = np.empty((T, B, Lq), f32)
    Sig_p = np.empty((T, B, Lq, Lq), f32)
    for t in range(T):
        y, A_t, C_t = obs[t], At[t], Ct[t]
        mu_p[t] = mu
        Sig_p[t] = Sig
        Ceff = np.einsum('op,bpl->bol', Rinv, C_t) if use_R else C_t
        G = np.einsum('bol,bok->blk', C_t, Ceff)
        M = I_L[None] + np.matmul(Sig, G)
        r = y - np.einsum('bol,bl->bo', C_t, mu)
        ctr = np.einsum('bol,bo->bl', Ceff, r)
        b1 = np.einsum('blk,bk->bl', Sig, ctr)
        sol = gj_solve(M, np.concatenate([Sig, b1[:, :, None]], -1))
        Sig_z = np.ascontiguousarray(sol[:, :, :Lq])
        mu_z = mu + sol[:, :, Lq]
        mu_f[t] = mu_z
        Sig_f[t] = Sig_z
        mu = np.einsum('blk,bk->bl', A_t, mu_z).astype(f32)
        Sig = (np.matmul(np.matmul(A_t, Sig_z), np.swapaxes(A_t, 1, 2))
               + Q.astype(f32)).astype(f32)
    outp = np.empty((T, B, Lq, Lq + 1), f32)
    mu_s = mu_f[T - 1].copy()
    Sig_s = Sig_f[T - 1].copy()
    outp[T - 1, :, :, 0] = mu_s
    outp[T - 1, :, :, 1:] = Sig_s
    for t in range(T - 2, -1, -1):
        ASf = np.matmul(At[t + 1], Sig_f[t])
        Jt = gj_solve(Sig_p[t + 1], ASf)
        mu_s = (mu_f[t] + np.einsum('bkl,bk->bl', Jt, mu_s - mu_p[t + 1])).astype(f32)
        JdS = np.einsum('bkl,bkm->blm', Jt, Sig_s - Sig_p[t + 1])
        Sig_s = (Sig_f[t] + np.matmul(JdS, Jt)).astype(f32)
        outp[t, :, :, 0] = mu_s
        outp[t, :, :, 1:] = Sig_s
    return outp


def kernel(obs, A, C, mu_1, Sigma_1, Q, R):
    obs = np.asarray(obs, dtype=np.float32)
    A = np.asarray(A, dtype=np.float32)
    C = np.asarray(C, dtype=np.float32)
    mu_1 = np.asarray(mu_1, dtype=np.float32)
    Sigma_1 = np.asarray(Sigma_1, dtype=np.float32)
    Q = np.asarray(Q, dtype=np.float32)
    R = np.asarray(R, dtype=np.float32)
    try:
        return kalman_bass(obs, A, C, mu_1, Sigma_1, Q, R)
    except Exception:
        import traceback
        traceback.print_exc()
        return _kalman_numpy(obs, A, C, mu_1, Sigma_1, Q, R)


# revision 18
# speedup vs baseline: 1.1574x; 1.0109x over previous
"""Kalman filter + RTS smoother kernel for Trainium2 (8 NeuronCores).

T=512 steps, B=512 batch, L=8 latent, O=16 obs. Data-parallel over B
(64/core); on each core the T axis is split into 16 chunks (2 on the
partition axis x 8 on the free axis, V=32 steps each) that run
concurrently, each with a W-step discarded warmup exploiting the
exponential forgetting (~0.4/step) of the Kalman recursions.

The 8 free-axis chunks are split into two independent instruction
streams: stream A's covariance chain runs on the Vector engine (DVE,
fp16 2x fast path), stream B's on the GpSimd/Pool engine, so the two
serial recursions pipeline against each other instead of serializing
on one engine. All 8x8 matmuls are emitted in ABt broadcast form
(unit innermost strides on both operands, exploiting the symmetry of
the covariance operands) with add-fold trees instead of TensorReduce.
The filter solve is a batched no-pivot Gauss-Jordan on the augmented
system [I + Sig G | Sig | Sig v] (divide ALU op, no reciprocal), which
yields Sig_z and the mu update in one elimination. All staging DRAM
(G/A/cy and the smoother inputs) is fp16 in h-major planes so every
per-step load/store is a single DMA.
"""
import sys
import numpy as np

for _p in ("/opt/trn_rl_repo",):
    if _p not in sys.path:
        sys.path.append(_p)

import concourse.bass as bass
import concourse.bacc as bacc
import concourse.mybir as mybir
from concourse.tile import TileContext
from concourse import bass_utils

F32 = mybir.dt.float32
F16 = mybir.dt.float16
AX = mybir.AxisListType
OP = mybir.AluOpType

L = 8
O = 16
LL = L * L
NB = 64
NCORE = 8
FW = 2 * LL + L          # fwd staging row: [G | A | cy] = 136
SW = 2 * (LL + L)        # smoother staging row: [Sf | muf | Sp(t+1) | mup(t+1)] = 144
PW = L * (L + 1)         # packed output row = 72


def build_kernel(T=512, NFA=8, NFB=0, W=6, TB=16,
                 pack_a="pool", pack_b="dve"):
    NF = NFA + NFB
    TH = T // 2
    V = TH // NF
    assert NF * V == TH and V >= W
    NSTEP = V + W
    ULEN = TH + 2 * W + 1
    SLEN = TH + 2 * W + 1

    nc = bacc.Bacc("TRN2", target_bir_lowering=False, debug=False,
                   num_devices=NCORE)

    a_in = nc.dram_tensor("a_in", (2, NB, TH, LL), F32, kind="ExternalInput")
    c_in = nc.dram_tensor("c_in", (2, NB, TH, O * L), F32, kind="ExternalInput")
    y_in = nc.dram_tensor("y_in", (2, NB, TH, O), F32, kind="ExternalInput")
    cons = nc.dram_tensor("cons", (128, 2 * LL), F16, kind="ExternalInput")
    csig = nc.dram_tensor("csig", (128, NFA * LL), F16, kind="ExternalInput")
    cmu = nc.dram_tensor("cmu", (128, NFA * L), F16, kind="ExternalInput")
    out = nc.dram_tensor("out", (T, NB, L, L + 1), F32, kind="ExternalOutput")

    FWD = nc.dram_tensor("fwd", (2, NB, ULEN, FW), F16, kind="Internal")
    SPFM = nc.dram_tensor("spfm", (2, NB, SLEN, SW), F16, kind="Internal")
    fwdv = FWD.ap().rearrange("h b u e -> (h b) u e")
    spfmv = SPFM.ap().rearrange("h b u e -> (h b) u e")

    VE = None  # set inside context

    def v4(flat_ap, nfx):
        return flat_ap.rearrange("p (f i k) -> p f i k", f=nfx, i=L, k=L)

    with TileContext(nc) as tc:
        with tc.tile_pool(name="const", bufs=1) as cpool:
            VE, PO = nc.vector, nc.gpsimd

            def eng(name):
                return VE if name == "dve" else PO

            # ---------------- constants ----------------
            CT = cpool.tile([128, 2 * LL], F16)
            nc.sync.dma_start(CT[:], cons.ap())
            Qv = CT[:, 0:LL].rearrange("p (i k) -> p i k", i=L, k=L)
            Iv = CT[:, LL:2 * LL].rearrange("p (i k) -> p i k", i=L, k=L)
            CS = cpool.tile([128, NFA * LL], F16)
            CM = cpool.tile([128, NFA * L], F16)
            nc.sync.dma_start(CS[:], csig.ap())
            nc.sync.dma_start(CM[:], cmu.ap())

            # ---------------- DRAM pads ----------------
            # FWD h=0 plane rows [0, W): zeros (chunk-0 warmup reads them)
            ZP = cpool.tile([128, W * FW], F16)
            VE.memset(ZP[:], 0.0)
            nc.sync.dma_start(
                FWD.ap()[0, :, 0:W, :],
                ZP[0:NB, :].rearrange("b (u e) -> b u e", u=W, e=FW))
            # FWD both planes row range [TH+W, TH+2W]: zero A pad for h=1
            # (t >= T => J=0 boundary); h=0 range is overwritten by the
            # dram-dram dup below.
            ZP2 = cpool.tile([128, (W + 1) * FW], F16)
            VE.memset(ZP2[:], 0.0)
            for h in range(2):
                nc.sync.dma_start(
                    FWD.ap()[h, :, TH + W:TH + 2 * W + 1, :],
                    ZP2[0:NB, :].rearrange("b (u e) -> b u e", u=W + 1, e=FW))
            # SPFM h=1 tail pad rows [TH+W, TH+2W]: [I | 0 | I | 0]
            SPAD = cpool.tile([128, (W + 1) * SW], F16)
            VE.memset(SPAD[:], 0.0)
            spadv = SPAD[:].rearrange("p (u e) -> p u e", u=W + 1, e=SW)
            for j in range(W + 1):
                PO.tensor_copy(spadv[:, j, 0:LL]
                               .rearrange("p (i k) -> p i k", i=L, k=L), Iv)
                PO.tensor_copy(spadv[:, j, LL + L:2 * LL + L]
                               .rearrange("p (i k) -> p i k", i=L, k=L), Iv)
            nc.sync.dma_start(
                SPFM.ap()[1, :, TH + W:TH + 2 * W + 1, :],
                SPAD[64:128, :].rearrange("b (u e) -> b u e", u=W + 1, e=SW))

            # ---------------- prep: G = C^T C, cy = C^T y, stage A ----------
            with tc.tile_pool(name="prep", bufs=2) as ppool:
                NIT = TH // TB
                for j in range(NIT):
                    E = VE if (j % 16) < 11 else PO
                    DQ = nc.sync
                    At = ppool.tile([128, TB * LL], F32, tag="pA")
                    Cr = ppool.tile([128, TB * O * L], F32, tag="pC")
                    Yr = ppool.tile([128, TB * O], F32, tag="pY")
                    DQ.dma_start(
                        At[:].rearrange("p (u e) -> p u e", u=TB, e=LL),
                        a_in.ap().rearrange("h b u e -> (h b) u e")
                        [:, j * TB:(j + 1) * TB, :])
                    DQ.dma_start(
                        Cr[:].rearrange("p (u e) -> p u e", u=TB, e=O * L),
                        c_in.ap().rearrange("h b u e -> (h b) u e")
                        [:, j * TB:(j + 1) * TB, :])
                    DQ.dma_start(
                        Yr[:].rearrange("p (u e) -> p u e", u=TB, e=O),
                        y_in.ap().rearrange("h b u e -> (h b) u e")
                        [:, j * TB:(j + 1) * TB, :])
                    PKF = ppool.tile([128, TB * FW], F16, tag="pF")
                    pkv = PKF[:].rearrange("p (u e) -> p u e", u=TB, e=FW)
                    # cast A into pack
                    E.tensor_copy(pkv[:, :, LL:2 * LL],
                                  At[:].rearrange("p (u e) -> p u e",
                                                  u=TB, e=LL))
                    # C^T (transposed cast): (u, l, o) <- (u, o, l)
                    CtT = ppool.tile([128, TB * O * L], F16, tag="pCt")
                    ctv = CtT[:].rearrange("p (u l o) -> p u l o",
                                           u=TB, l=L, o=O)
                    E.tensor_copy(ctv,
                                  Cr[:].rearrange("p (u o l) -> p u o l",
                                                  u=TB, o=O, l=L)
                                  .rearrange("p u o l -> p u l o"))
                    Y16 = ppool.tile([128, TB * O], F16, tag="pY16")
                    E.tensor_copy(Y16[:], Yr[:])
                    # G mults: per u, (l, m, o) = Ct[l,o] * Ct[m,o]
                    GT = ppool.tile([128, TB * L * L * O], F16, tag="pG")
                    gtv = GT[:].rearrange("p (u l m o) -> p u l m o",
                                          u=TB, l=L, m=L, o=O)
                    for u in range(TB):
                        E.tensor_tensor(
                            gtv[:, u],
                            ctv[:, u].unsqueeze(2).broadcast_to((128, L, L, O)),
                            ctv[:, u].unsqueeze(1).broadcast_to((128, L, L, O)),
                            OP.mult)
                    gk = GT[:].rearrange("p (x o) -> p x o", o=O)
                    E.tensor_tensor(gk[:, :, 0:8], gk[:, :, 0:8],
                                    gk[:, :, 8:16], OP.add)
                    E.tensor_tensor(gk[:, :, 0:4], gk[:, :, 0:4],
                                    gk[:, :, 4:8], OP.add)
                    E.tensor_tensor(gk[:, :, 0:2], gk[:, :, 0:2],
                                    gk[:, :, 2:4], OP.add)
                    gkv = GT[:].rearrange("p (u x o) -> p u x o",
                                          u=TB, x=LL, o=O)
                    E.tensor_tensor(pkv[:, :, 0:LL],
                                    gkv[:, :, :, 0], gkv[:, :, :, 1], OP.add)
                    # cy: (u, l, o) = Ct[l,o] * y[o]
                    CYT = ppool.tile([128, TB * L * O], F16, tag="pcy")
                    cyv = CYT[:].rearrange("p (u l o) -> p u l o",
                                           u=TB, l=L, o=O)
                    E.tensor_tensor(
                        cyv, ctv,
                        Y16[:].rearrange("p (u o) -> p u o", u=TB, o=O)
                        .unsqueeze(2).broadcast_to((128, TB, L, O)),
                        OP.mult)
                    ck = CYT[:].rearrange("p (x o) -> p x o", o=O)
                    E.tensor_tensor(ck[:, :, 0:8], ck[:, :, 0:8],
                                    ck[:, :, 8:16], OP.add)
                    E.tensor_tensor(ck[:, :, 0:4], ck[:, :, 0:4],
                                    ck[:, :, 4:8], OP.add)
                    E.tensor_tensor(ck[:, :, 0:2], ck[:, :, 0:2],
                                    ck[:, :, 2:4], OP.add)
                    ckv = CYT[:].rearrange("p (u l o) -> p u l o",
                                           u=TB, l=L, o=O)
                    E.tensor_tensor(pkv[:, :, 2 * LL:2 * LL + L],
                                    ckv[:, :, :, 0], ckv[:, :, :, 1], OP.add)
                    DQ.dma_start(
                        fwdv[:, W + j * TB:W + (j + 1) * TB, :], pkv)

            # dram-dram dups across the h boundary:
            # FWD h=1 front pad [0, W) <- h=0 rows [TH, TH+W)
            nc.sync.dma_start(FWD.ap()[1, :, 0:W, :],
                              FWD.ap()[0, :, TH:TH + W, :])
            # FWD h=0 tail rows [TH+W, TH+2W] <- h=1 rows [W, 2W]
            nc.sync.dma_start(FWD.ap()[0, :, TH + W:TH + 2 * W + 1, :],
                              FWD.ap()[1, :, W:2 * W + 1, :])

            # ---------------- helper emitters ----------------
            def mm_abt(E, dstv, tmp, x4, y4, nfx):
                """dst[f,a,c] = sum_k x4[f,a,k] * y4[f,c,k]."""
                tv = tmp[:].rearrange("p (f a c k) -> p f a c k",
                                      f=nfx, a=L, c=L, k=L)
                for i in range(nfx):
                    E.tensor_tensor(
                        tv[:, i],
                        x4[:, i].unsqueeze(2).broadcast_to((128, L, L, L)),
                        y4[:, i].unsqueeze(1).broadcast_to((128, L, L, L)),
                        OP.mult)
                tk = tmp[:, 0:nfx * 512].rearrange("p (x k) -> p x k", k=L)
                E.tensor_tensor(tk[:, :, 0:4], tk[:, :, 0:4],
                                tk[:, :, 4:8], OP.add)
                E.tensor_tensor(tk[:, :, 0:2], tk[:, :, 0:2],
                                tk[:, :, 2:4], OP.add)
                E.tensor_tensor(
                    dstv,
                    tk[:, :, 0].rearrange("p (f a c) -> p f a c",
                                          f=nfx, a=L, c=L),
                    tk[:, :, 1].rearrange("p (f a c) -> p f a c",
                                          f=nfx, a=L, c=L),
                    OP.add)

            def mv(E, dst3, tmp, x4, v3, nfx, kind="Av"):
                """dst[f,a] = sum_k X[f,a,k] v[f,k] (Av) or X[f,k,a] v[f,k].
                v3 is a (p, f, k) view."""
                xv = x4 if kind == "Av" else x4.rearrange("p f k l -> p f l k")
                vv = v3.unsqueeze(2).broadcast_to((128, nfx, L, L))
                tv = tmp[:, 0:nfx * LL].rearrange("p (f a k) -> p f a k",
                                                  f=nfx, a=L, k=L)
                E.tensor_tensor(tv, xv, vv, OP.mult)
                E.tensor_tensor(tv[:, :, :, 0:4], tv[:, :, :, 0:4],
                                tv[:, :, :, 4:8], OP.add)
                E.tensor_tensor(tv[:, :, :, 0:2], tv[:, :, :, 0:2],
                                tv[:, :, :, 2:4], OP.add)
                E.tensor_tensor(dst3, tv[:, :, :, 0], tv[:, :, :, 1], OP.add)

            def gj(E, augv, prt, t2t, crt, rct, nfx, wid):
                prtv = prt[:].rearrange("p (r f j) -> p r f j",
                                        r=2, f=nfx, j=wid - 1)
                rctv = rct[:].rearrange("p (r f j) -> p r f j",
                                        r=2, f=nfx, j=1)
                pend = None
                for pp in range(L):
                    jw = wid - 1 - pp
                    prv = prtv[:, pp % 2][:, :, 0:jw]
                    rcv = rctv[:, pp % 2]
                    with nc.allow_low_precision(
                            reason="fp16 pivot reciprocal, pivots are O(1)"):
                        VE.reciprocal(rcv, augv[:, :, pp, pp:pp + 1])
                    E.tensor_tensor(
                        prv, augv[:, :, pp, pp + 1:],
                        rcv.broadcast_to((128, nfx, jw)), OP.mult)
                    if pend is not None:
                        E.tensor_copy(augv[:, :, pend[0], pend[0] + 1:],
                                      pend[1])
                    crv = crt[:].rearrange("p (f i j) -> p f i j",
                                           f=nfx, i=L, j=wid - 1)[:, :, :, 0:jw]
                    E.tensor_copy(crv,
                                  augv[:, :, :, pp:pp + 1]
                                  .broadcast_to((128, nfx, L, jw)))
                    t2v = t2t[:].rearrange("p (f i j) -> p f i j",
                                           f=nfx, i=L, j=wid - 1)[:, :, :, 0:jw]
                    E.tensor_tensor(
                        t2v, crv,
                        prv.unsqueeze(2).broadcast_to((128, nfx, L, jw)),
                        OP.mult)
                    E.tensor_tensor(augv[:, :, :, pp + 1:],
                                    augv[:, :, :, pp + 1:], t2v, OP.subtract)
                    pend = (pp, prv)
                E.tensor_copy(augv[:, :, L - 1, L:], pend[1])

            # ---------------- stream descriptors ----------------
            # Each stream is fully self-contained on one compute engine and
            # one DMA-issue queue (ACT / PE sequencers are otherwise idle),
            # so the two serial recursions never cross-stall. Loads run two
            # steps ahead of the recursion.
            streams = [
                dict(nfx=NFA, off=0, E=VE, Eo=PO, name="a", DQ=nc.sync),
            ]

            with tc.tile_pool(name="work", bufs=3) as pool, \
                 tc.tile_pool(name="work2", bufs=2) as pool2:

                def ft_load(st, s):
                    nfx, off = st["nfx"], st["off"]
                    FT = pool.tile([128, nfx * FW], F16,
                                   tag="FT" + st["name"],
                                   name="FT" + st["name"], bufs=4)
                    st["DQ"].dma_start(
                        FT[:].rearrange("p (f e) -> p f e", f=nfx, e=FW),
                        fwdv[:, off * V + s:
                             off * V + s + (nfx - 1) * V + 1:V, :])
                    st["ftq"].append(FT)

                # ============ FORWARD FILTER ============
                for st in streams:
                    nfx = st["nfx"]
                    st["SIG"] = cpool.tile([128, nfx * LL], F16,
                                           name="SIG0" + st["name"])
                    st["MU"] = cpool.tile([128, nfx * L], F16,
                                          name="MU0" + st["name"])
                    st["E"].memset(st["SIG"][:], 0.0)
                    st["E"].memset(st["MU"][:], 0.0)
                    st["ftq"] = []
                    ft_load(st, 0)
                    ft_load(st, 1)

                for s in range(NSTEP):
                    for st in streams:
                        nfx = st["nfx"]
                        if s + 2 < NSTEP:
                            ft_load(st, s + 2)
                        FT = st["ftq"].pop(0)
                        ftv = FT[:].rearrange("p (f e) -> p f e", f=nfx, e=FW)
                        st["Gt4"] = ftv[:, :, 0:LL].rearrange(
                            "p f (i k) -> p f i k", i=L, k=L)
                        st["At4"] = ftv[:, :, LL:2 * LL].rearrange(
                            "p f (i k) -> p f i k", i=L, k=L)
                        st["CYf"] = ftv[:, :, 2 * LL:2 * LL + L]

                    if s == W:
                        stA = streams[0]
                        VE.tensor_tensor(stA["SIG"][:], stA["SIG"][:], CS[:],
                                         OP.add)
                        VE.tensor_tensor(stA["MU"][:], stA["MU"][:], CM[:],
                                         OP.add)

                    for st in streams:
                        nfx = st["nfx"]
                        nm = st["name"]
                        st["AUG"] = pool2.tile([128, nfx * L * 17], F16,
                                               tag="AUG" + nm, name="AUG" + nm)
                        st["augv"] = st["AUG"][:].rearrange(
                            "p (f i j) -> p f i j", f=nfx, i=L, j=17)
                        st["TMP"] = pool2.tile([128, nfx * 512], F16,
                                               tag="TMP" + nm, name="TMP" + nm)
                        st["PR"] = pool.tile([128, nfx * 2 * 16], F16,
                                             tag="PR" + nm, name="PR" + nm)
                        st["T2"] = pool2.tile([128, nfx * L * 16], F16,
                                              tag="T2" + nm, name="T2" + nm)
                        st["MT"] = pool.tile([128, nfx * LL], F16,
                                             tag="MT" + nm, name="MT" + nm)
                        st["RC"] = pool.tile([128, nfx * 2], F16,
                                             tag="RC" + nm, name="RC" + nm)
                        st["CR"] = pool.tile([128, nfx * L * 16], F16,
                                             tag="CR" + nm, name="CR" + nm)

                    # aug assembly: cols 8:16 = Sig, col 16 = Sig cy + mu
                    # (mu_z = M^-1 (Sig cy + mu) -- push-through identity)
                    for st in streams:
                        nfx, Eo = st["nfx"], st["Eo"]
                        sig4 = v4(st["SIG"][:], nfx)
                        Eo.tensor_copy(st["augv"][:, :, :, L:2 * L], sig4)
                        mv(Eo, st["augv"][:, :, :, 16], st["MT"], sig4,
                           st["CYf"], nfx)
                        Eo.tensor_tensor(
                            st["augv"][:, :, :, 16],
                            st["augv"][:, :, :, 16],
                            st["MU"][:].rearrange("p (f a) -> p f a",
                                                  f=nfx, a=L), OP.add)

                    # chain: SG mm -> aug cols 0:8, then +I
                    for st in streams:
                        nfx, E = st["nfx"], st["E"]
                        mm_abt(E, st["augv"][:, :, :, 0:L], st["TMP"],
                               v4(st["SIG"][:], nfx), st["Gt4"], nfx)
                        diag = st["AUG"][:].rearrange(
                            "p (f e) -> p f e", f=nfx, e=L * 17)[:, :, 0:L * 17 - 1:18]
                        E.tensor_scalar(diag, diag, 1.0, None, OP.add)

                    # chain: GJ -> [I | Sig_z | mu_z]
                    for st in streams:
                        gj(st["E"], st["augv"], st["PR"], st["T2"],
                           st["CR"], st["RC"], st["nfx"], 17)

                    # mu' = A mu_z
                    for st in streams:
                        nfx, Eo = st["nfx"], st["Eo"]
                        MUn = pool.tile([128, nfx * L], F16,
                                        tag="MUn" + st["name"],
                                        name="MUn" + st["name"])
                        mv(Eo, MUn[:].rearrange("p (f a) -> p f a",
                                                f=nfx, a=L),
                           st["MT"], st["At4"], st["augv"][:, :, :, 16], nfx)
                        st["MUn"] = MUn

                    # chain: ASZ = A Sigz, SIG' = ASZ A^T + Q
                    for st in streams:
                        nfx, E = st["nfx"], st["E"]
                        sigz4 = st["augv"][:, :, :, L:2 * L]
                        ASZ = pool.tile([128, nfx * LL], F16,
                                        tag="ASZ" + st["name"],
                                        name="ASZ" + st["name"])
                        mm_abt(E, v4(ASZ[:], nfx), st["TMP"], st["At4"],
                               sigz4, nfx)
                        SIGn = pool.tile([128, nfx * LL], F16,
                                         tag="SIGn" + st["name"],
                                         name="SIGn" + st["name"])
                        mm_abt(E, v4(SIGn[:], nfx), st["TMP"],
                               v4(ASZ[:], nfx), st["At4"], nfx)
                        E.tensor_tensor(
                            v4(SIGn[:], nfx), v4(SIGn[:], nfx),
                            Qv.unsqueeze(1).broadcast_to((128, nfx, L, L)),
                            OP.add)
                        st["SIGn"] = SIGn

                    # pack [Sf | muf | Sp(t+1) | mup(t+1)] and store
                    if s >= W:
                        for st in streams:
                            nfx, off, E = st["nfx"], st["off"], st["Eo"]
                            PACK = pool.tile([128, nfx * SW], F16,
                                             tag="PK" + st["name"],
                                             name="PK" + st["name"])
                            pkv = PACK[:].rearrange("p (f e) -> p f e",
                                                    f=nfx, e=SW)
                            E.tensor_copy(
                                pkv[:, :, 0:LL].rearrange(
                                    "p f (i k) -> p f i k", i=L, k=L),
                                st["augv"][:, :, :, L:2 * L])
                            E.tensor_copy(pkv[:, :, LL:LL + L],
                                          st["augv"][:, :, :, 16])
                            E.tensor_copy(
                                pkv[:, :, LL + L:2 * LL + L].rearrange(
                                    "p f (i k) -> p f i k", i=L, k=L),
                                v4(st["SIGn"][:], nfx))
                            E.tensor_copy(
                                pkv[:, :, 2 * LL + L:SW],
                                st["MUn"][:].rearrange("p (f a) -> p f a",
                                                       f=nfx, a=L))
                            st["DQ"].dma_start(
                                spfmv[:, off * V + s:
                                      off * V + s + (nfx - 1) * V + 1:V, :],
                                pkv)
                            if st["off"] == 0 and W <= s <= 2 * W:
                                st["DQ"].dma_start(
                                    SPFM.ap()[0, :, TH + s, :],
                                    PACK[64:128, 0:SW])

                    for st in streams:
                        st["SIG"] = st["SIGn"]
                        st["MU"] = st["MUn"]

                # ============ BACKWARD SMOOTHER ============
                def bwd_load(st, r):
                    nfx, off, nm = st["nfx"], st["off"], st["name"]
                    SFT = pool.tile([128, nfx * SW], F16, tag="SF" + nm,
                                    name="SF" + nm, bufs=4)
                    row = off * V + V + 2 * W - 1 - r
                    st["DQ"].dma_start(
                        SFT[:].rearrange("p (f e) -> p f e", f=nfx, e=SW),
                        spfmv[:, row:row + (nfx - 1) * V + 1:V, :])
                    At1 = pool.tile([128, nfx * LL], F16, tag="Ab" + nm,
                                    name="Ab" + nm, bufs=4)
                    st["DQ"].dma_start(
                        At1[:].rearrange("p (f e) -> p f e", f=nfx, e=LL),
                        fwdv[:, row + 1:row + 1 + (nfx - 1) * V + 1:V,
                             LL:2 * LL])
                    st["bq"].append((SFT, At1))

                for st in streams:
                    nfx, off = st["nfx"], st["off"]
                    INIT = cpool.tile([128, nfx * (LL + L)], F16,
                                      name="INIT" + st["name"])
                    st["DQ"].dma_start(
                        INIT[:].rearrange("p (f e) -> p f e",
                                          f=nfx, e=LL + L),
                        spfmv[:, off * V + V + 2 * W:
                              off * V + V + 2 * W + (nfx - 1) * V + 1:V,
                              0:LL + L])
                    iv = INIT[:].rearrange("p (f e) -> p f e", f=nfx, e=LL + L)
                    st["SIGSv"] = iv[:, :, 0:LL].rearrange(
                        "p f (i k) -> p f i k", i=L, k=L)
                    st["MUSv"] = iv[:, :, LL:LL + L]
                    st["bq"] = []
                    bwd_load(st, 0)
                    bwd_load(st, 1)

                for r in range(NSTEP):
                    for st in streams:
                        nfx = st["nfx"]
                        nm = st["name"]
                        if r + 2 < NSTEP:
                            bwd_load(st, r + 2)
                        SFT, At1 = st["bq"].pop(0)
                        sfv = SFT[:].rearrange("p (f e) -> p f e", f=nfx, e=SW)
                        st["Sf4"] = sfv[:, :, 0:LL].rearrange(
                            "p f (i k) -> p f i k", i=L, k=L)
                        st["muf"] = sfv[:, :, LL:LL + L]
                        st["Sp4"] = sfv[:, :, LL + L:2 * LL + L].rearrange(
                            "p f (i k) -> p f i k", i=L, k=L)
                        st["mup"] = sfv[:, :, 2 * LL + L:SW]
                        st["At4"] = v4(At1[:], nfx)

                        st["AUG"] = pool2.tile([128, nfx * L * 16], F16,
                                               tag="AUG" + nm,
                                               name="AUGb" + nm)
                        st["augv"] = st["AUG"][:].rearrange(
                            "p (f i j) -> p f i j", f=nfx, i=L, j=16)
                        st["TMP"] = pool2.tile([128, nfx * 512], F16,
                                               tag="TMP" + nm,
                                               name="TMPb" + nm)
                        st["PR"] = pool.tile([128, nfx * 2 * 15], F16,
                                             tag="PR" + nm, name="PRb" + nm)
                        st["T2"] = pool2.tile([128, nfx * L * 15], F16,
                                              tag="T2" + nm, name="T2b" + nm)
                        st["MT"] = pool.tile([128, nfx * LL], F16,
                                             tag="MT" + nm, name="MTb" + nm)
                        st["RC"] = pool.tile([128, nfx * 2], F16,
                                             tag="RC" + nm, name="RCb" + nm)
                        st["CR"] = pool.tile([128, nfx * L * 15], F16,
                                             tag="CR" + nm, name="CRb" + nm)

                    # DS, DM, aug Sp copy
                    for st in streams:
                        nfx, E = st["nfx"], st["Eo"]
                        DS = pool.tile([128, nfx * LL], F16,
                                       tag="DS" + st["name"],
                                       name="DS" + st["name"])
                        E.tensor_tensor(v4(DS[:], nfx), st["SIGSv"],
                                        st["Sp4"], OP.subtract)
                        st["DS4"] = v4(DS[:], nfx)
                        DM = pool.tile([128, nfx * L], F16,
                                       tag="DM" + st["name"],
                                       name="DM" + st["name"])
                        E.tensor_tensor(
                            DM[:].rearrange("p (f a) -> p f a", f=nfx, a=L),
                            st["MUSv"], st["mup"], OP.subtract)
                        st["DM"] = DM
                        E.tensor_copy(st["augv"][:, :, :, 0:L], st["Sp4"])

                    # chain: RHS = A Sf -> aug cols 8:16
                    for st in streams:
                        mm_abt(st["E"], st["augv"][:, :, :, L:2 * L],
                               st["TMP"], st["At4"], st["Sf4"], st["nfx"])

                    # chain: GJ (width 16) -> jt4 = inv(Sp) (A Sf) = J^T
                    for st in streams:
                        gj(st["E"], st["augv"], st["PR"], st["T2"],
                           st["CR"], st["RC"], st["nfx"], 16)

                    # MUS' = muf + jt4^T DM
                    for st in streams:
                        nfx, E = st["nfx"], st["Eo"]
                        MUSn = pool.tile([128, nfx * L], F16,
                                         tag="MUSn" + st["name"],
                                         name="MUSn" + st["name"])
                        mus3 = MUSn[:].rearrange("p (f a) -> p f a",
                                                 f=nfx, a=L)
                        mv(E, mus3, st["MT"], st["augv"][:, :, :, L:2 * L],
                           st["DM"][:].rearrange("p (f a) -> p f a",
                                                 f=nfx, a=L),
                           nfx, kind="Atv")
                        E.tensor_tensor(mus3, st["muf"], mus3, OP.add)
                        st["MUSn"] = MUSn

                    # chain: Jc = jt4^T, T3 = Jc DS (ABt), SIGS' = T3 Jc^T + Sf
                    for st in streams:
                        nfx, E = st["nfx"], st["E"]
                        nm = st["name"]
                        jt4 = st["augv"][:, :, :, L:2 * L]
                        JC = pool.tile([128, nfx * LL], F16, tag="JC" + nm,
                                       name="JC" + nm)
                        E.tensor_copy(v4(JC[:], nfx),
                                      jt4.rearrange("p f k l -> p f l k"))
                        T3 = pool.tile([128, nfx * LL], F16, tag="T3" + nm,
                                       name="T3" + nm)
                        mm_abt(E, v4(T3[:], nfx), st["TMP"], v4(JC[:], nfx),
                               st["DS4"], nfx)
                        SIGSn = pool.tile([128, nfx * LL], F16,
                                          tag="SGn" + nm, name="SGn" + nm)
                        mm_abt(E, v4(SIGSn[:], nfx), st["TMP"],
                               v4(T3[:], nfx), v4(JC[:], nfx), nfx)
                        E.tensor_tensor(v4(SIGSn[:], nfx), v4(SIGSn[:], nfx),
                                        st["Sf4"], OP.add)
                        st["SIGSn"] = SIGSn

                    # pack fp32 output + store
                    if r >= W:
                        for st in streams:
                            nfx, off, E = st["nfx"], st["off"], st["Eo"]
                            PK = pool.tile([128, nfx * PW], F32,
                                           tag="PKo" + st["name"],
                                           name="PKo" + st["name"])
                            pko = PK[:].rearrange("p (f l j) -> p f l j",
                                                  f=nfx, l=L, j=L + 1)
                            E.tensor_copy(
                                pko[:, :, :, 0],
                                st["MUSn"][:].rearrange("p (f a) -> p f a",
                                                        f=nfx, a=L))
                            E.tensor_copy(pko[:, :, :, 1:L + 1],
                                          v4(st["SIGSn"][:], nfx))
                            tb0 = off * V + V + W - 1 - r
                            for h in range(2):
                                tb = h * TH + tb0
                                st["DQ"].dma_start(
                                    out.ap()[tb:tb + (nfx - 1) * V + 1:V,
                                             :, :, :]
                                    .rearrange("f b l j -> b f (l j)"),
                                    PK[h * NB:(h + 1) * NB, :]
                                    .rearrange("b (f e) -> b f e",
                                               f=nfx, e=PW))

                    for st in streams:
                        st["SIGSv"] = v4(st["SIGSn"][:], st["nfx"])
                        st["MUSv"] = st["MUSn"][:].rearrange(
                            "p (f a) -> p f a", f=st["nfx"], a=L)

    nc.compile()
    return nc


_CACHE = {}


def get_kernel(T=512, NFA=8, NFB=0, W=6):
    key = (T, NFA, NFB, W)
    if key not in _CACHE:
        _CACHE[key] = build_kernel(T=T, NFA=NFA, NFB=NFB, W=W)
    return _CACHE[key]


def make_in_maps(obs, A, C, mu_1, Sigma_1, Q, R, NFA=8):
    f32, f16 = np.float32, np.float16
    T = obs.shape[0]
    TH = T // 2
    cons = np.zeros((128, 2 * LL), f16)
    cons[:, :LL] = np.asarray(Q, f32).ravel()[None].astype(f16)
    cons[:, LL:] = np.eye(L, dtype=f32).ravel()[None].astype(f16)
    csig = np.zeros((128, NFA * LL), f16)
    cmu = np.zeros((128, NFA * L), f16)
    csig[0:NB, 0:LL] = (np.asarray(Sigma_1, f32)
                        - np.asarray(Q, f32)).ravel()[None].astype(f16)
    cmu[0:NB, 0:L] = np.asarray(mu_1, f32)[None].astype(f16)
    in_maps = []
    for c in range(NCORE):
        sl = slice(c * NB, (c + 1) * NB)
        Ac = np.asarray(A[sl], f32).reshape(NB, 2, TH, LL)
        Ac = np.ascontiguousarray(Ac.transpose(1, 0, 2, 3))
        Cc = np.asarray(C[sl], f32).reshape(NB, 2, TH, O * L)
        Cc = np.ascontiguousarray(Cc.transpose(1, 0, 2, 3))
        Yc = np.asarray(obs[:, sl], f32).reshape(2, TH, NB, O)
        Yc = np.ascontiguousarray(Yc.transpose(0, 2, 1, 3))
        in_maps.append({
            "a_in": Ac, "c_in": Cc, "y_in": Yc,
            "cons": cons, "csig": csig, "cmu": cmu,
        })
    return in_maps


def kalman_bass(obs, A, C, mu_1, Sigma_1, Q, R, T=512, nc=None):
    assert obs.shape[0] == T and obs.shape[1] == NB * NCORE
    if not np.allclose(R, np.eye(O), atol=1e-6):
        raise ValueError("general R not supported on device path")
    if nc is None:
        nc = get_kernel(T)
    in_maps = make_in_maps(obs, A, C, mu_1, Sigma_1, Q, R)
    res = bass_utils.run_bass_kernel_spmd(nc, in_maps,
                                          core_ids=list(range(NCORE)))
    return np.concatenate([res.results[c]["out"] for c in range(NCORE)],
                          axis=1)


# ---------------------------------------------------------------------------
# Slow numpy fallback (used only if the device path fails)
def _kalman_numpy(obs, A, C, mu_1, Sigma_1, Q, R):
    f32 = np.float32
    T, B, Oq = obs.shape
    Lq = mu_1.shape[0]
    At = np.ascontiguousarray(np.swapaxes(A, 0, 1)).astype(f32)
    Ct = np.ascontiguousarray(np.swapaxes(C, 0, 1)).astype(f32)
    I_L = np.eye(Lq, dtype=f32)

    def gj_solve(M, RHS):
        n = M.shape[1]
        Maug = np.concatenate([M, RHS], axis=-1).astype(f32)
        for p in range(n):
            recip = (f32(1.0) / Maug[:, p, p]).astype(f32)
            Maug[:, p, :] = Maug[:, p, :] * recip[:, None]
            col = Maug[:, :, p].copy()
            col[:, p] = 0.0
            Maug = (Maug - col[:, :, None] * Maug[:, p, None, :]).astype(f32)
        return Maug[:, :, n:]

    Rinv = np.linalg.inv(R.astype(np.float64)).astype(f32)
    use_R = not np.allclose(R, np.eye(Oq, dtype=f32))
    mu = np.broadcast_to(mu_1.astype(f32), (B, Lq)).copy()
    Sig = np.broadcast_to(Sigma_1.astype(f32), (B, Lq, Lq)).copy()
    mu_f = np.empty((T, B, Lq), f32)
    Sig_f = np.empty((T, B, Lq, Lq), f32)
    mu_p = np.empty((T, B, Lq), f32)
    Sig_p = np.empty((T, B, Lq, Lq), f32)
    for t in range(T):
        y, A_t, C_t = obs[t], At[t], Ct[t]
        mu_p[t] = mu
        Sig_p[t] = Sig
        Ceff = np.einsum('op,bpl->bol', Rinv, C_t) if use_R else C_t
        G = np.einsum('bol,bok->blk', C_t, Ceff)
        M = I_L[None] + np.matmul(Sig, G)
        r = y - np.einsum('bol,bl->bo', C_t, mu)
        ctr = np.einsum('bol,bo->bl', Ceff, r)
        b1 = np.einsum('blk,bk->bl', Sig, ctr)
        sol = gj_solve(M, np.concatenate([Sig, b1[:, :, None]], -1))
        Sig_z = np.ascontiguousarray(sol[:, :, :Lq])
        mu_z = mu + sol[:, :, Lq]
        mu_f[t] = mu_z
        Sig_f[t] = Sig_z
        mu = np.einsum('blk,bk->bl', A_t, mu_z).astype(f32)
        Sig = (np.matmul(np.matmul(A_t, Sig_z), np.swapaxes(A_t, 1, 2))
               + Q.astype(f32)).astype(f32)
    outp = np.empty((T, B, Lq, Lq + 1), f32)
    mu_s = mu_f[T - 1].copy()
    Sig_s = Sig_f[T - 1].copy()
    outp[T - 1, :, :, 0] = mu_s
    outp[T - 1, :, :, 1:] = Sig_s
    for t in range(T - 2, -1, -1):
        ASf = np.matmul(At[t + 1], Sig_f[t])
        Jt = gj_solve(Sig_p[t + 1], ASf)
        mu_s = (mu_f[t] + np.einsum('bkl,bk->bl', Jt, mu_s - mu_p[t + 1])).astype(f32)
        JdS = np.einsum('bkl,bkm->blm', Jt, Sig_s - Sig_p[t + 1])
        Sig_s = (Sig_f[t] + np.matmul(JdS, Jt)).astype(f32)
        outp[t, :, :, 0] = mu_s
        outp[t, :, :, 1:] = Sig_s
    return outp


def kernel(obs, A, C, mu_1, Sigma_1, Q, R):
    obs = np.asarray(obs, dtype=np.float32)
    A = np.asarray(A, dtype=np.float32)
    C = np.asarray(C, dtype=np.float32)
    mu_1 = np.asarray(mu_1, dtype=np.float32)
    Sigma_1 = np.asarray(Sigma_1, dtype=np.float32)
    Q = np.asarray(Q, dtype=np.float32)
    R = np.asarray(R, dtype=np.float32)
    try:
        return kalman_bass(obs, A, C, mu_1, Sigma_1, Q, R)
    except Exception:
        import traceback
        traceback.print_exc()
        return _kalman_numpy(obs, A, C, mu_1, Sigma_1, Q, R)
